# revision 74
# baseline (speedup 1.0000x reference)
"""CategoryAwareDAHEAD Trainium2 kernel (8-core SPMD, data-parallel over ROIs).

V2 design
---------
* Host prep: ins_features cast to bf16 and pre-transposed per core to a
  feature-major layout [128p, 16g, 64n, 49w] (feature d = 16p+g, i.e. the
  same PERM as the baseline opT layout).  W1/W2/W3/Wd/biases cast to bf16
  (W1 rows PERM-permuted).  rel-err of bf16 features+weights vs f32
  reference measured at 9.2e-5 (gate is 2e-2).
* Stream: 16 chunks of [128, 64*49] bf16 (~0.8MB each) on the sync queue;
  per chunk: 7x7-mean on the Pool engine into opT f32, a bf16 scaled copy
  opTb (x1/49) for the EA branch, EA layer-1 PSUM accumulation (bf16
  matmuls), and a PE transpose into sample-major feat64 f32.
  opT is deliberately left unscaled (cosines are scale-invariant).
* MGRM phase-1 (softmax stats, threshold chain, accept weights, local
  gathers) only needs logits/labels: issued first, fully hidden under the
  stream.  Chain identical to baseline (th <- max(th,(th+m)/2) monotone
  reformulation).
* Cross-core reduction: ONE bf16 AllReduce of [128, 1024]: cols 0..1007 =
  per-chunk transposed partials [ps|pt|bw] (16 chunks x 63 cols), col 1008
  = loss_ea partial.  Final cosine math is done from the 63x63 Gram blocks
  (cos is computed from unnormalized Grams + diag norms), replicated on
  every core.
* EA layers: bf16 matmuls, LayerNorm fused as mean/E[x2] reductions + one
  Relu activation with AP scale/bias; only {Exp, Ln, Square} activation
  functions are used so a single act table set covers the whole kernel.

KLEVEL env (debug bisection): pool|ea|mgrm|nocc|full
"""

import os
import sys

for _p in ("/opt/trn_rl_repo", "/root/.axon_site/_ro/trn_rl_repo"):
    if _p not in sys.path:
        sys.path.insert(0, _p)

import numpy as np
import ml_dtypes

import concourse.bacc as bacc
import concourse.mybir as mybir
import concourse.tile as tile
from concourse import bass_utils
from concourse.masks import make_identity, make_upper_triangular, make_lower_triangular

F32 = mybir.dt.float32
BF16 = mybir.dt.bfloat16
FP8 = mybir.dt.float8e4
I32 = mybir.dt.int32
AX = mybir.AxisListType
OP = mybir.AluOpType
ACT = mybir.ActivationFunctionType
BFnp = ml_dtypes.bfloat16
F8np = ml_dtypes.float8_e4m3

NCORES = 8
N, NS, C, D = 512, 256, 21, 2048
NL = N // NCORES          # 64 rows per core
H1 = 1024
DIN = D + C               # 2069
S = 32                    # per-class sequence table length (max count is 18)
THR0, MOM, EPS, LN_EPS, MGRM_W = 0.1, 0.5, 1e-8, 1e-5, 1.0

# feature permutation: new index f holds original d = (f % 128) * 16 + f // 128
PERM = (np.arange(D) % 128) * 16 + np.arange(D) // 128


def _build():
    lv = os.environ.get("KLEVEL", "full")
    do_mgrm = lv in ("mgrm", "nocc", "full")
    do_ea = lv in ("ea", "nocc", "full")
    do_final = lv in ("nocc", "full")

    nc = bacc.Bacc("TRN2", target_bir_lowering=False, debug=False,
                   num_devices=NCORES)

    # ---------------- DRAM I/O ----------------
    # pair-major feature stream [128pair x 25 groups, 2048f]: pooling is
    # done on the PE as selector matmuls, keeping the DVE free for the
    # latency-critical MGRM phase-1 chain
    featp_dr = nc.dram_tensor("featP", [128, 25 * D], FP8,
                              kind="ExternalInput")
    enw_dr = nc.dram_tensor("Enw", [128, 25 * NL], FP8, kind="ExternalInput")
    lg_dr = nc.dram_tensor("logits_full", [N, C], F32, kind="ExternalInput")
    lgT_dr = nc.dram_tensor("logitsT_loc", [C, NL], BF16, kind="ExternalInput")
    lab_dr = nc.dram_tensor("labels_in", [NS], I32, kind="ExternalInput")
    dom_dr = nc.dram_tensor("dom_shard", [NL, 1], I32, kind="ExternalInput")
    sels_dr = nc.dram_tensor("sel_src", [NS, NL], F32, kind="ExternalInput")
    selt_dr = nc.dram_tensor("sel_tgt", [NS, NL], F32, kind="ExternalInput")
    w1_dr = nc.dram_tensor("W1p", [DIN, H1], BF16, kind="ExternalInput")
    w2_dr = nc.dram_tensor("W2in", [H1, H1], BF16, kind="ExternalInput")
    w3_dr = nc.dram_tensor("W3in", [H1, H1], BF16, kind="ExternalInput")
    wd_dr = nc.dram_tensor("Wdin", [H1, 1], BF16, kind="ExternalInput")
    b1_dr = nc.dram_tensor("b1in", [1, H1], BF16, kind="ExternalInput")
    b2_dr = nc.dram_tensor("b2in", [1, H1], BF16, kind="ExternalInput")
    b3_dr = nc.dram_tensor("b3in", [1, H1], BF16, kind="ExternalInput")
    bd_dr = nc.dram_tensor("bdin", [1, 1], BF16, kind="ExternalInput")
    out_dr = nc.dram_tensor("out_loss", [1, 2], F32, kind="ExternalOutput")

    featp3 = featp_dr.ap().rearrange("p (j x) -> p j x", j=25)  # x = 2048

    with tile.TileContext(nc) as tc:
        with (
            tc.tile_pool(name="consts", bufs=1) as cst,
            tc.tile_pool(name="insb", bufs=1) as insb,
            tc.tile_pool(name="featp", bufs=4) as featp,
            tc.tile_pool(name="persist", bufs=1) as per,
            tc.tile_pool(name="wpool", bufs=18) as wp,
            tc.tile_pool(name="wpool2", bufs=16) as wp2,
            tc.tile_pool(name="work", bufs=1) as wk,
            tc.tile_pool(name="pps", bufs=3, space="PSUM") as pps,
            tc.tile_pool(name="pph", bufs=1, space="PSUM") as pph,
            tc.tile_pool(name="dram", bufs=1, space="DRAM") as drp,
        ):
            # ---------------- constants ----------------
            id128 = cst.tile([128, 128], F32, tag="id128")
            make_identity(nc, id128[:])
            ut128 = cst.tile([128, 128], F32, tag="ut128")
            make_upper_triangular(nc, ut128[:], val=1.0, diag=True)
            lt128 = cst.tile([128, 128], F32, tag="lt128")
            make_lower_triangular(nc, lt128[:], val=1.0, diag=True)
            ones = cst.tile([128, 128], F32, tag="ones")
            nc.gpsimd.memset(ones[:], 1.0)
            onesbf = cst.tile([1, NL], BF16, tag="onesbf")
            nc.gpsimd.memset(onesbf[:], 1.0)
            iotaS = cst.tile([128, S], F32, tag="iotaS")   # 1..S per partition
            nc.gpsimd.iota(iotaS[:], [[1, S]], base=1, channel_multiplier=0,
                           allow_small_or_imprecise_dtypes=True)
            iota21 = cst.tile([128, C], F32, tag="iota21")  # 0..20
            nc.gpsimd.iota(iota21[:], [[1, C]], base=0, channel_multiplier=0,
                           allow_small_or_imprecise_dtypes=True)
            iota63 = cst.tile([128, 63], F32, tag="iota63")  # col index 0..62
            nc.gpsimd.iota(iota63[:], [[1, 63]], base=0, channel_multiplier=0,
                           allow_small_or_imprecise_dtypes=True)
            pc21 = cst.tile([128, 1], F32, tag="pc21")       # 21 + partition
            nc.gpsimd.iota(pc21[:], [[1, 1]], base=21, channel_multiplier=1,
                           allow_small_or_imprecise_dtypes=True)
            pc42 = cst.tile([128, 1], F32, tag="pc42")       # 42 + partition
            nc.gpsimd.iota(pc42[:], [[1, 1]], base=42, channel_multiplier=1,
                           allow_small_or_imprecise_dtypes=True)


            # ---------------- small input DMAs ----------------
            # sync queue: lg + labels first (phase-1 needs them), then stream
            lg_sb = insb.tile([128, 4 * C], F32, tag="lg")       # [128, 84]
            nc.sync.dma_start(
                lg_sb[:].rearrange("p (c l) -> p c l", c=4),
                lg_dr.ap().rearrange("(c p) l -> p c l", p=128))
            lab_i = insb.tile([128, 2], I32, tag="labi")
            nc.sync.dma_start(lab_i[:], lab_dr.ap().rearrange(
                "(c p) -> p c", p=128))
            lab_f = insb.tile([128, 2], F32, tag="labf")
            nc.vector.tensor_copy(lab_f[:], lab_i[:])

            # ============ MGRM phase 1: logits-only math ============
            eall = None
            counts = None
            if do_mgrm:
                # per-chunk softmax stats (chunks of 128 samples; 0,1=src 2,3=tgt)
                E_ch, OHP_ch, mlOHP_ch = [], [], []
                for ch in range(4):
                    lg_c = lg_sb[:, C * ch:C * (ch + 1)]
                    mx = wk.tile([128, 1], F32, tag=f"mx{ch}")
                    nc.vector.tensor_reduce(mx[:], lg_c, axis=AX.X, op=OP.max)
                    E = wk.tile([128, C], F32, tag=f"E{ch}")
                    nc.vector.tensor_scalar(out=E[:], in0=lg_c, scalar1=mx[:],
                                            scalar2=None, op0=OP.is_equal)
                    negmx = wk.tile([128, 1], F32, tag=f"nmx{ch}")
                    nc.vector.tensor_scalar_mul(negmx[:], mx[:], -1.0)
                    scr = wk.tile([128, C], F32, tag=f"scr{ch}")
                    den = wk.tile([128, 1], F32, tag=f"den{ch}")
                    nc.scalar.activation(scr[:], lg_c, ACT.Exp, bias=negmx[:],
                                         scale=1.0, accum_out=den[:])
                    ml = wk.tile([128, 1], F32, tag=f"ml{ch}")
                    nc.vector.reciprocal(ml[:], den[:])
                    E_ch.append(E)

                    # position of each sample within its domain's class list
                    dom = ch // 2   # 0 = src, 1 = tgt
                    P_ps = pps.tile([128, C], F32, tag="pps")
                    if ch % 2 == 0:
                        nc.tensor.matmul(P_ps[:], ut128[:], E[:], start=True,
                                         stop=True)
                    else:
                        nc.tensor.matmul(P_ps[:], ones[:], E_ch[2 * dom][:],
                                         start=True, stop=False)
                        nc.tensor.matmul(P_ps[:], ut128[:], E[:], start=False,
                                         stop=True)
                    pos = wk.tile([128, 1], F32, tag=f"pos{ch}")
                    posscr = wk.tile([128, C], F32, tag=f"poss{ch}")
                    nc.vector.tensor_tensor(posscr[:], P_ps[:], E[:],
                                            op=OP.mult)
                    nc.vector.tensor_reduce(pos[:], posscr[:], axis=AX.X,
                                            op=OP.add)
                    OHP = wk.tile([128, S], F32, tag=f"OHP{ch}")
                    nc.vector.tensor_scalar(out=OHP[:], in0=iotaS[:],
                                            scalar1=pos[:], scalar2=None,
                                            op0=OP.is_equal)
                    mlOHP = wk.tile([128, S], F32, tag=f"mlO{ch}")
                    nc.vector.tensor_scalar(out=mlOHP[:], in0=iotaS[:],
                                            scalar1=pos[:], scalar2=ml[:],
                                            op0=OP.is_equal, op1=OP.mult)
                    OHP_ch.append(OHP)
                    mlOHP_ch.append(mlOHP)

                sels_sb = insb.tile([128, 2 * NL], F32, tag="sels")
                nc.scalar.dma_start(
                    sels_sb[:].rearrange("p (c n) -> p c n", c=2),
                    sels_dr.ap().rearrange("(c p) n -> p c n", p=128))
                selt_sb = insb.tile([128, 2 * NL], F32, tag="selt")
                nc.scalar.dma_start(
                    selt_sb[:].rearrange("p (c n) -> p c n", c=2),
                    selt_dr.ap().rearrange("(c p) n -> p c n", p=128))

                # tables T[21, 2S]: cols 0..S-1 src, S..2S-1 tgt (dom-major)
                T_all = wk.tile([C, 2 * S], F32, tag="Tall")
                for dom in range(2):
                    T_ps = pps.tile([C, S], F32, tag="pps")
                    nc.tensor.matmul(T_ps[:], E_ch[2 * dom][:],
                                     mlOHP_ch[2 * dom][:], start=True,
                                     stop=False)
                    nc.tensor.matmul(T_ps[:], E_ch[2 * dom + 1][:],
                                     mlOHP_ch[2 * dom + 1][:], start=False,
                                     stop=True)
                    nc.vector.tensor_copy(T_all[:, S * dom:S * (dom + 1)],
                                          T_ps[:])

                # ------------- sequential threshold chain -------------
                # th history keeps the serial loop at 2 DVE ops/step; all 32
                # accept masks come from ONE vectorized is_ge afterwards
                T3 = T_all[:].rearrange("p (d s) -> p d s", d=2)
                thh = wk.tile([C, 2 * (S + 1)], F32, tag="thh")
                th3 = thh[:].rearrange("p (s d) -> p s d", s=S + 1)
                nc.gpsimd.memset(th3[:, 0, :], THR0)
                tmp = wk.tile([C, 2], F32, tag="chtmp")
                for s in range(S):
                    nc.vector.tensor_tensor(tmp[:], T3[:, :, s], th3[:, s, :],
                                            op=OP.add)
                    nc.vector.scalar_tensor_tensor(
                        out=th3[:, s + 1, :], in0=tmp[:], scalar=0.5,
                        in1=th3[:, s, :], op0=OP.mult, op1=OP.max)
                A = wk.tile([C, 2 * S], F32, tag="A")
                A3 = A[:].rearrange("p (d s) -> p d s", d=2)
                nc.vector.tensor_tensor(
                    A3[:].rearrange("p d s -> p s d"), T3[:].rearrange(
                        "p d s -> p s d"), th3[:, 0:S, :], op=OP.is_ge)

                # per-domain accept-index math -> wtab [21, 2S]
                # w_j = prod_{i>=j, accepted} (1/J_i): gated reverse product
                # scan (no Ln/Exp -> no act-table switches)
                gt = wk.tile([C, 2 * S], F32, tag="gt")
                gt3 = gt[:].rearrange("p (d s) -> p d s", d=2)
                for dom in range(2):
                    A_dom = A[:, S * dom:S * (dom + 1)]
                    A_T = wk.tile([S, C], F32, tag=f"AT{dom}")
                    at_ps = pps.tile([S, C], F32, tag="pps")
                    nc.tensor.transpose(at_ps[:], A_dom, id128[0:C, 0:C])
                    nc.vector.tensor_copy(A_T[:], at_ps[:])
                    J_ps = pps.tile([C, S], F32, tag="pps")
                    nc.tensor.matmul(J_ps[:], A_T[:], ut128[0:S, 0:S],
                                     start=True, stop=True)
                    jc = wk.tile([C, S], F32, tag=f"jc{dom}")
                    nc.vector.tensor_scalar(out=jc[:], in0=J_ps[:], scalar1=1.0,
                                            scalar2=None, op0=OP.max)
                    rj = wk.tile([C, S], F32, tag=f"rj{dom}")
                    nc.vector.reciprocal(rj[:], jc[:])
                    # g = A ? 1/J : 1  =  (1/J)*A + (1 - A)
                    am = wk.tile([C, S], F32, tag=f"am{dom}")
                    nc.vector.tensor_scalar(out=am[:], in0=A_dom, scalar1=-1.0,
                                            scalar2=1.0, op0=OP.mult,
                                            op1=OP.add)
                    gd = gt3[:, dom, :]
                    nc.vector.tensor_tensor(gd, rj[:], A_dom, op=OP.mult)
                    nc.vector.tensor_tensor(gd, gd, am[:], op=OP.add)
                # suffix product scan (Pool engine), then wtab = pw * A
                pw = wk.tile([C, 2 * S], F32, tag="pw")
                pw3 = pw[:].rearrange("p (d s) -> p d s", d=2)
                nc.vector.tensor_copy(pw3[:, :, S - 1], gt3[:, :, S - 1])
                for s in range(S - 2, -1, -1):
                    nc.vector.tensor_tensor(pw3[:, :, s], gt3[:, :, s],
                                            pw3[:, :, s + 1], op=OP.mult)
                wtab = wk.tile([C, 2 * S], F32, tag="wtab")
                nc.vector.tensor_tensor(wtab[:], pw[:], A[:], op=OP.mult)

                # ---------- per-sample weights, local gathers ----------
                # eall [64, 63+1]: cols 0:21 src EW, 21:42 tgt EW, 42:63 label
                eall = wk.tile([NL, 64], BF16, tag="eall")
                for dom in range(2):
                    sel = sels_sb if dom == 0 else selt_sb
                    ewl_ps = pps.tile([NL, C], F32, tag="pps")
                    for cc in range(2):
                        ch = 2 * dom + cc
                        ET = wk.tile([C, 128], F32, tag=f"ET{ch}")
                        et_ps = pps.tile([C, 128], F32, tag="pps")
                        nc.tensor.transpose(et_ps[:], E_ch[ch][:], id128[:])
                        nc.vector.tensor_copy(ET[:], et_ps[:])
                        G_ps = pps.tile([128, S], F32, tag="pps")
                        nc.tensor.matmul(G_ps[:], ET[:],
                                         wtab[:, S * dom:S * (dom + 1)],
                                         start=True, stop=True)
                        ws = wk.tile([128, 1], F32, tag=f"ws{ch}")
                        wscr = wk.tile([128, S], F32, tag=f"wscr{ch}")
                        nc.vector.tensor_tensor(wscr[:], G_ps[:],
                                                OHP_ch[ch][:], op=OP.mult)
                        nc.vector.tensor_reduce(ws[:], wscr[:], axis=AX.X,
                                                op=OP.add)
                        EW = wk.tile([128, C], F32, tag=f"EW{ch}")
                        nc.vector.tensor_scalar(out=EW[:], in0=E_ch[ch][:],
                                                scalar1=ws[:], scalar2=None,
                                                op0=OP.mult)
                        nc.tensor.matmul(ewl_ps[:],
                                         sel[:, NL * cc:NL * (cc + 1)],
                                         EW[:], start=(cc == 0), stop=(cc == 1))
                    nc.vector.tensor_copy(eall[:, C * dom:C * (dom + 1)],
                                          ewl_ps[:])

                # labels onehot + counts + local label gather
                elab_l_ps = pps.tile([NL, C], F32, tag="pps")
                cnt_ps = pps.tile([C, 1], F32, tag="pps")
                for cc in range(2):
                    Elab = wk.tile([128, C], F32, tag=f"Elab{cc}")
                    nc.vector.tensor_scalar(out=Elab[:], in0=iota21[:],
                                            scalar1=lab_f[:, cc:cc + 1],
                                            scalar2=None, op0=OP.is_equal)
                    nc.tensor.matmul(cnt_ps[:], Elab[:], ones[:, 0:1],
                                     start=(cc == 0), stop=(cc == 1))
                    nc.tensor.matmul(elab_l_ps[:],
                                     sels_sb[:, NL * cc:NL * (cc + 1)],
                                     Elab[:], start=(cc == 0), stop=(cc == 1))
                nc.vector.tensor_copy(eall[:, 2 * C:3 * C], elab_l_ps[:])
                counts = wk.tile([C, 1], F32, tag="counts")
                nc.vector.tensor_copy(counts[:], cnt_ps[:])

            # ============ feature stream + pool + EA-L1 + feat64 ============
            opTb = per.tile([128, 16 * NL], BF16, tag="opTb")  # x(1/49), bf16
            opTb3 = opTb[:].rearrange("p (g n) -> p g n", g=16)
            feat64 = per.tile([NL, D], BF16, tag="feat64")     # sample-major

            bounce = drp.tile([128, 1008], FP8, tag="bounce")
            prT = per.tile([128, 16 * 63], FP8, tag="prT")
            id128b = cst.tile([128, 128], BF16, tag="id128b")
            nc.vector.tensor_copy(id128b[:], id128[:])

            def prt_chunks(gs):
                for g in gs:
                    pr_ps = pps.tile([128, 63], F32, tag="pps")
                    nc.tensor.matmul(pr_ps[:], feat64[:, 128 * g:128 * (g + 1)],
                                     eall[:, 0:63], start=True, stop=True)
                    nc.vector.tensor_copy(prT[:, 63 * g:63 * (g + 1)], pr_ps[:])

            enw_sb = insb.tile([128, 25 * NL], FP8, tag="enw")
            nc.scalar.dma_start(enw_sb[:], enw_dr[:, :])
            lgT_sb = insb.tile([C, NL], BF16, tag="lgT")
            nc.scalar.dma_start(lgT_sb[:], lgT_dr[:, :])
            b1_sb = insb.tile([1, H1], BF16, tag="b1")
            nc.scalar.dma_start(b1_sb[:], b1_dr[:, :])
            b2_sb = insb.tile([1, H1], BF16, tag="b2")
            nc.scalar.dma_start(b2_sb[:], b2_dr[:, :])
            b3_sb = insb.tile([1, H1], BF16, tag="b3")
            nc.scalar.dma_start(b3_sb[:], b3_dr[:, :])
            bd_sb = insb.tile([1, 1], BF16, tag="bd")
            nc.scalar.dma_start(bd_sb[:], bd_dr[:, :])
            wd_sb = insb.tile([128, 8], BF16, tag="wd")
            nc.scalar.dma_start(
                wd_sb[:].rearrange("p (c o) -> p c o", c=8),
                wd_dr.ap().rearrange("(c p) o -> p c o", p=128))
            dom_i = insb.tile([NL, 1], I32, tag="domi")
            nc.scalar.dma_start(dom_i[:], dom_dr[:, :])
            dom_f = insb.tile([NL, 1], F32, tag="domf")
            nc.vector.tensor_copy(dom_f[:], dom_i[:])

            # --- stream: 25 pair-major chunks, pooled on the PE ---
            # pool_ps accumulates the full [64n, 2048f] pooled sums (4 psum
            # banks); L1 runs post-stream so its 2 banks never coexist with
            # a partial pool.
            pool_ps = pph.tile([NL, D], F32, tag="hbig")
            # 12 double-group chunks via fp8 DoubleRow matmuls (0.5 cyc/row:
            # out += E_j^T @ X_j + E_j+1^T @ X_j+1 in one instruction),
            # plus a final single group
            MM2 = mybir.MatmulPerfMode.DoubleRow
            for jp in range(12):
                ftp = featp.tile([128, 2 * D], FP8, tag="ftp")
                nc.sync.dma_start(
                    ftp[:].rearrange("p (g x) -> p g x", g=2),
                    featp3[:, 2 * jp:2 * jp + 2, :])
                ftp3 = ftp[:].rearrange("p (g x) -> p g x", g=2)
                lhs3 = enw_sb[:, 2 * NL * jp:2 * NL * (jp + 1)].rearrange(
                    "p (g n) -> p g n", g=2)
                for q in range(4):
                    nc.tensor.matmul(pool_ps[:, 512 * q:512 * (q + 1)],
                                     lhs3,
                                     ftp3[:, :, 512 * q:512 * (q + 1)],
                                     start=(jp == 0), stop=False,
                                     perf_mode=MM2)
            ftp = featp.tile([128, 2 * D], FP8, tag="ftp")
            nc.sync.dma_start(ftp[0:128, 0:D], featp3[:, 24, :])
            for q in range(4):
                nsl = slice(512 * q, 512 * (q + 1))
                nc.tensor.matmul(pool_ps[:, nsl],
                                 enw_sb[:, NL * 24:NL * 25],
                                 ftp[0:128, nsl],
                                 start=False, stop=True)

            # W1 + W2/W3 prefetch (act queue drains these mid-stream)
            w1ts = {}
            if do_ea:
                for g in range(16):
                    w1t = wp.tile([128, H1], BF16, tag="w")
                    w1ts[g] = w1t
                    nc.scalar.dma_start(w1t[:],
                                        w1_dr[128 * g:128 * (g + 1), :])
                wt16 = wp.tile([128, H1], BF16, tag="w")
                nc.scalar.dma_start(wt16[0:C, :], w1_dr[D:D + C, :])
                wpre = {}
                for wi, w_dr in enumerate((w2_dr, w3_dr)):
                    for kc in range(8):
                        wpt = wp2.tile([128, H1], BF16, tag="w2")
                        wpre[(wi, kc)] = wpt
                        # the fp8 stream is short: both W2 and W3 ride the
                        # sync queue after it, keeping the act queue clear
                        # for the post-pool copies
                        nc.sync.dma_start(wpt[:],
                                          w_dr[128 * kc:128 * (kc + 1), :])

            # pooled sums -> feat64 (sample-major, bf16) per psum quadrant,
            # with the AR-gating prototype partials pipelined right behind
            for q in range(4):
                nsl = slice(512 * q, 512 * (q + 1))
                if q % 2 == 0:
                    nc.scalar.activation(feat64[:, nsl], pool_ps[:, nsl],
                                         ACT.Copy)
                else:
                    nc.vector.tensor_copy(feat64[:, nsl], pool_ps[:, nsl])
                if do_mgrm:
                    for g2 in range(4 * q, 4 * q + 4, 2):
                        pr2_ps = pps.tile([128, 126], F32, tag="pps")
                        for k in range(2):
                            g = g2 + k
                            nc.tensor.matmul(pr2_ps[:, 63 * k:63 * (k + 1)],
                                             feat64[:, 128 * g:128 * (g + 1)],
                                             eall[:, 0:63], start=True,
                                             stop=True)
                        if (g2 // 2) % 2 == 0:
                            nc.vector.tensor_copy(
                                prT[:, 63 * g2:63 * (g2 + 2)], pr2_ps[:])
                        else:
                            nc.scalar.activation(
                                prT[:, 63 * g2:63 * (g2 + 2)], pr2_ps[:],
                                ACT.Copy)
            if do_mgrm:
                nc.sync.dma_start(bounce[:], prT[:])
            # opT-orientation bf16 copies for the EA layer-1 lhsT
            for g in range(16):
                tp2_ps = pps.tile([128, NL], BF16, tag="pps")
                nc.tensor.transpose(tp2_ps[:],
                                    feat64[:, 128 * g:128 * (g + 1)],
                                    id128b[0:NL, 0:NL])
                nc.scalar.activation(opTb3[:, g, :], tp2_ps[:], ACT.Copy,
                                     scale=1.0 / 49.0)
            if do_ea:
                h_big = pph.tile([NL, D], F32, tag="hbig")
                h_ps = h_big[:, 0:H1]
                for half in range(2):
                    nsl = slice(512 * half, 512 * (half + 1))
                    nc.tensor.matmul(h_ps[:, nsl], onesbf[:], b1_sb[:, nsl],
                                     start=True, stop=False)
                for half in range(2):
                    nsl = slice(512 * half, 512 * (half + 1))
                    nc.tensor.matmul(h_ps[:, nsl], lgT_sb[:], wt16[0:C, nsl],
                                     start=False, stop=False)
                for g in range(16):
                    for half in range(2):
                        nsl = slice(512 * half, 512 * (half + 1))
                        nc.tensor.matmul(h_ps[:, nsl], opTb3[:, g, :],
                                         w1ts[g][:, nsl], start=False,
                                         stop=(g == 15))

            # ============ EA layers ============
            U16 = mybir.dt.uint16

            def rsqrt16(y, x, shape, tg):
                """y = x^-0.5 on DVE: bf16 bit-trick seed + 2 Newton steps.
                (no rsqrt opcode on DVE; act-table Sqrt would thrash the
                single {exp,ln,square} table set.  The DVE ALU datapath is
                fp32 internally so only 16-bit ints survive bit tricks.)"""
                xb = wk.tile(shape, BF16, tag=f"rsqb{tg}")
                nc.vector.tensor_copy(xb[:], x[:])
                t16 = wk.tile(shape, U16, tag=f"rsqs{tg}")
                nc.vector.tensor_scalar(out=t16[:], in0=xb[:].bitcast(U16),
                                        scalar1=1, scalar2=None,
                                        op0=OP.logical_shift_right)
                y16 = wk.tile(shape, U16, tag=f"rsqy{tg}")
                nc.vector.tensor_scalar(out=y16[:], in0=t16[:], scalar1=-1.0,
                                        scalar2=float(0x5F37), op0=OP.mult,
                                        op1=OP.add)
                t1 = wk.tile(shape, F32, tag=f"rsqt{tg}")
                nc.vector.tensor_copy(y[:], y16[:].bitcast(BF16))
                for _ in range(2):
                    nc.vector.tensor_tensor(t1[:], y[:], y[:], op=OP.mult)
                    nc.vector.tensor_tensor(t1[:], t1[:], x[:], op=OP.mult)
                    nc.vector.tensor_scalar(out=t1[:], in0=t1[:], scalar1=-0.5,
                                            scalar2=1.5, op0=OP.mult,
                                            op1=OP.add)
                    nc.vector.tensor_tensor(y[:], y[:], t1[:], op=OP.mult)

            def ln_relu(h_in, scaled):
                """relu(layernorm(h_in)) -> f32 [NL, H1].

                scaled=False drops the 1/std factor: relu(c*x) = c*relu(x)
                and the NEXT LayerNorm removes any per-row scale exactly
                (biases are zero in this problem), so only the last LN
                before the sigmoid needs the true rstd."""
                musum = wk.tile([NL, 1], F32, tag="mu")
                nc.vector.tensor_reduce(musum[:], h_in[:], axis=AX.X,
                                        op=OP.add)
                mu = wk.tile([NL, 1], F32, tag="mus")
                nc.vector.tensor_scalar_mul(mu[:], musum[:], 1.0 / H1)
                h = wk.tile([NL, H1], F32, tag="h")
                if not scaled:
                    nmu = wk.tile([NL, 1], F32, tag="nmu")
                    nc.vector.tensor_scalar_mul(nmu[:], musum[:], -1.0 / H1)
                    nc.scalar.activation(h[:], h_in[:], ACT.Relu, bias=nmu[:])
                    return h
                sqs = wk.tile([NL, H1], F32, tag="sqs")
                q = wk.tile([NL, 1], F32, tag="q")
                nc.scalar.activation(sqs[:], h_in[:], ACT.Square,
                                     accum_out=q[:])
                nmu2e = wk.tile([NL, 1], F32, tag="nmu2e")
                nc.vector.tensor_scalar(out=nmu2e[:], in0=mu[:], scalar1=mu[:],
                                        scalar2=-1.0, op0=OP.mult, op1=OP.mult)
                nc.vector.tensor_scalar(out=nmu2e[:], in0=nmu2e[:],
                                        scalar1=float(LN_EPS), scalar2=None,
                                        op0=OP.add)
                ve = wk.tile([NL, 1], F32, tag="ve")
                nc.vector.tensor_scalar(out=ve[:], in0=q[:], scalar1=1.0 / H1,
                                        scalar2=nmu2e[:], op0=OP.mult,
                                        op1=OP.add)
                rstd = wk.tile([NL, 1], F32, tag="rstd")
                rsqrt16(rstd, ve, [NL, 1], "ln")
                nb = wk.tile([NL, 1], F32, tag="nb")
                nc.vector.tensor_scalar(out=nb[:], in0=mu[:], scalar1=rstd[:],
                                        scalar2=-1.0, op0=OP.mult, op1=OP.mult)
                nc.scalar.activation(h[:], h_in[:], ACT.Relu, scale=rstd[:],
                                     bias=nb[:])
                return h

            def transpose_h(h):
                """[NL, H1] f32 -> [128, 8*NL] bf16 (chunk j = features 128j..)"""
                hT = wk.tile([128, 8 * NL], BF16, tag="hT")
                for half in range(2):
                    ht_ps = pps.tile([128, 4 * NL], F32, tag="pps")
                    for j in range(4):
                        jj = 4 * half + j
                        nc.tensor.transpose(ht_ps[:, NL * j:NL * (j + 1)],
                                            h[:, 128 * jj:128 * (jj + 1)],
                                            id128[0:NL, 0:NL])
                    nc.vector.tensor_copy(
                        hT[:, 4 * NL * half:4 * NL * (half + 1)], ht_ps[:])
                return hT

            lea_ps = None
            if do_ea:
                h = ln_relu(h_ps[:], scaled=True)
                for li_w, (w_dr, b_sb) in enumerate(((w2_dr, b2_sb),
                                                     (w3_dr, b3_sb))):
                    hT = transpose_h(h)
                    h_big = pph.tile([NL, D], F32, tag="hbig")
                    h_ps = h_big[:, 0:H1]
                    for half in range(2):
                        nsl = slice(512 * half, 512 * (half + 1))
                        nc.tensor.matmul(h_ps[:, nsl], onesbf[:], b_sb[:, nsl],
                                         start=True, stop=False)
                    for kc in range(8):
                        wt = wpre[(li_w, kc)]
                        for half in range(2):
                            nsl = slice(512 * half, 512 * (half + 1))
                            nc.tensor.matmul(h_ps[:, nsl],
                                             hT[:, NL * kc:NL * (kc + 1)],
                                             wt[:, nsl],
                                             start=False,
                                             stop=(kc == 7))
                    h = ln_relu(h_ps[:], scaled=True)

                h3T = transpose_h(h)
                zd_ps = pps.tile([NL, 1], F32, tag="pps")
                nc.tensor.matmul(zd_ps[:], onesbf[:], bd_sb[:],
                                 start=True, stop=False)
                for kc in range(8):
                    nc.tensor.matmul(zd_ps[:], h3T[:, NL * kc:NL * (kc + 1)],
                                     wd_sb[:, kc:kc + 1], start=False,
                                     stop=(kc == 7))
                # z = sigmoid(zd) = 1/(1+exp(-zd))
                enz = wk.tile([NL, 1], F32, tag="enz")
                nc.scalar.activation(enz[:], zd_ps[:], ACT.Exp, scale=-1.0)
                zden = wk.tile([NL, 1], F32, tag="zden")
                nc.vector.tensor_scalar(out=zden[:], in0=enz[:], scalar1=1.0,
                                        scalar2=None, op0=OP.add)
                z = wk.tile([NL, 1], F32, tag="z")
                nc.vector.reciprocal(z[:], zden[:])
                # softplus(-z) = ln(1 + exp(-z))
                emz = wk.tile([NL, 1], F32, tag="emz")
                nc.scalar.activation(emz[:], z[:], ACT.Exp, scale=-1.0)
                sp = wk.tile([NL, 1], F32, tag="sp")
                nc.scalar.activation(sp[:], emz[:], ACT.Ln,
                                     bias=ones[0:NL, 0:1])
                omy = wk.tile([NL, 1], F32, tag="omy")
                nc.vector.tensor_scalar(out=omy[:], in0=dom_f[:], scalar1=-1.0,
                                        scalar2=1.0, op0=OP.mult, op1=OP.add)
                li_t = wk.tile([NL, 1], F32, tag="li")
                nc.vector.scalar_tensor_tensor(out=li_t[:], in0=z[:],
                                               scalar=omy[:], in1=sp[:],
                                               op0=OP.mult, op1=OP.add)
                # per-core loss_ea partial: reduced on the HOST (data-parallel
                # gather), so the AllReduce is not gated on the EA branch
                lea_ps = pps.tile([1, 1], F32, tag="pps")
                nc.tensor.matmul(lea_ps[:], li_t[:], ones[0:NL, 0:1],
                                 start=True, stop=True)


            if do_final:
                # ---- cross-core reduce: ReduceScatter + AllGather ----
                # (cheaper than AllReduce: no 1.875x single-instr penalty)
                bounce_out = drp.tile([128, 1008], FP8, tag="bounce_out",
                                      addr_space="Shared")
                if lv == "nocc":
                    nc.sync.dma_start(bounce_out[:], bounce[:])
                else:
                    bmid = drp.tile([16, 1008], FP8, tag="bmid")
                    nc.gpsimd.collective_compute(
                        "ReduceScatter", OP.add,
                        replica_groups=[list(range(NCORES))],
                        ins=[bounce[:].opt()], outs=[bmid[:].opt()])
                    nc.gpsimd.collective_compute(
                        "AllGather", OP.bypass,
                        replica_groups=[list(range(NCORES))],
                        ins=[bmid[:].opt()], outs=[bounce_out[:].opt()])
                XT = per.tile([128, 16 * 63], FP8, tag="XT")
                nc.sync.dma_start(XT[:], bounce_out[:])

                # ---------------- final (replicated) ----------------
                # full Gram G[63,63] of the stacked [ps|pt|bw] prototypes;
                # diag = squared norms, blocks = cross inner products
                G_ps = pps.tile([63, 63], F32, tag="pps")
                for g in range(16):
                    base = 63 * g
                    nc.tensor.matmul(G_ps[:], XT[:, base:base + 63],
                                     XT[:, base:base + 63],
                                     start=(g == 0), stop=(g == 15))
                G_sb = wk.tile([63, 63], F32, tag="Gsb")
                nc.vector.tensor_copy(G_sb[:], G_ps[:])
                # realign pt rows (21..41) and bw rows (42..62) down to
                # partitions 0..20 with shifted-identity matmuls
                Gpt_ps = pps.tile([C, 63], F32, tag="pps")
                nc.tensor.matmul(Gpt_ps[:], id128[0:63, C:2 * C], G_sb[:],
                                 start=True, stop=True)
                Gbw_ps = pps.tile([C, 63], F32, tag="pps")
                nc.tensor.matmul(Gbw_ps[:], id128[0:63, 2 * C:3 * C], G_sb[:],
                                 start=True, stop=True)
                # diag extraction masks on partitions 0..20
                mpt = wk.tile([C, 63], F32, tag="mpt")
                nc.vector.tensor_scalar(out=mpt[:], in0=iota63[0:C, :],
                                        scalar1=pc21[0:C, :], scalar2=None,
                                        op0=OP.is_equal)
                mbw = wk.tile([C, 63], F32, tag="mbw")
                nc.vector.tensor_scalar(out=mbw[:], in0=iota63[0:C, :],
                                        scalar1=pc42[0:C, :], scalar2=None,
                                        op0=OP.is_equal)
                # d3 cols: 0 = ps, 1 = pt, 2 = bw squared norms
                d3 = wk.tile([C, 3], F32, tag="d3")
                dsc = wk.tile([C, 63], F32, tag="dsc")
                nc.vector.tensor_tensor(dsc[:], G_sb[0:C, :], id128[0:C, 0:63],
                                        op=OP.mult)
                nc.vector.tensor_reduce(d3[:, 0:1], dsc[:], axis=AX.X,
                                        op=OP.add)
                nc.vector.tensor_tensor(dsc[:], Gpt_ps[:], mpt[:], op=OP.mult)
                nc.vector.tensor_reduce(d3[:, 1:2], dsc[:], axis=AX.X,
                                        op=OP.add)
                nc.vector.tensor_tensor(dsc[:], Gbw_ps[:], mbw[:], op=OP.mult)
                nc.vector.tensor_reduce(d3[:, 2:3], dsc[:], axis=AX.X,
                                        op=OP.add)
                # rn3 = 1/max(sqrt(d), eps) = max(d, eps^2)^-0.5
                dmx3 = wk.tile([C, 3], F32, tag="dmx3")
                nc.vector.tensor_scalar(out=dmx3[:], in0=d3[:],
                                        scalar1=float(EPS) ** 2, scalar2=None,
                                        op0=OP.max)
                rn3 = wk.tile([C, 3], F32, tag="rn3")
                rsqrt16(rn3, dmx3, [C, 3], "rn")

                cosP = wk.tile([C, C], F32, tag="cosP")
                nc.vector.tensor_scalar(out=cosP[:],
                                        in0=G_sb[0:C, C:2 * C],
                                        scalar1=rn3[:, 0:1], scalar2=None,
                                        op0=OP.mult)
                cosB = wk.tile([C, C], F32, tag="cosB")
                nc.vector.tensor_scalar(out=cosB[:], in0=Gbw_ps[:, C:2 * C],
                                        scalar1=rn3[:, 2:3], scalar2=None,
                                        op0=OP.mult)
                absd = wk.tile([C, C], F32, tag="absd")
                nc.vector.tensor_tensor(absd[:], cosB[:], cosP[:],
                                        op=OP.subtract)
                nc.scalar.activation(absd[:], absd[:], ACT.Abs)
                # cem = (counts > 0) with class 0 zeroed; wv = rn_t * cem
                cem = wk.tile([C, 1], F32, tag="cem")
                nc.vector.tensor_scalar(out=cem[:], in0=counts[:], scalar1=0.0,
                                        scalar2=None, op0=OP.is_gt)
                nc.gpsimd.memset(cem[0:1, :], 0.0)
                wv = wk.tile([C, 1], F32, tag="wv")
                nc.vector.tensor_tensor(wv[:], rn3[:, 1:2], cem[:],
                                        op=OP.mult)
                # sum over j = 1..20 only (reference drops row/col 0)
                nc.gpsimd.memset(absd[0:1, :], 0.0)
                s1_ps = pps.tile([C, 1], F32, tag="pps")
                nc.tensor.matmul(s1_ps[:], absd[:], ones[0:C, 0:1],
                                 start=True, stop=True)
                s1 = wk.tile([C, 1], F32, tag="s1")
                nc.vector.tensor_copy(s1[:], s1_ps[:])
                tot_ps = pps.tile([1, 1], F32, tag="pps")
                nc.tensor.matmul(tot_ps[:], s1[:], wv[:], start=True, stop=True)
                nm_ps = pps.tile([1, 1], F32, tag="pps")
                nc.tensor.matmul(nm_ps[:], cem[:], ones[0:C, 0:1],
                                 start=True, stop=True)
                nm_sb = wk.tile([1, 1], F32, tag="nmsb")
                nc.vector.tensor_copy(nm_sb[:], nm_ps[:])
                rnm = wk.tile([1, 1], F32, tag="rnm")
                nc.vector.reciprocal(rnm[:], nm_sb[:])

                res = wk.tile([1, 2], F32, tag="res")
                nc.vector.tensor_scalar(out=res[:, 0:1], in0=tot_ps[:],
                                        scalar1=rnm[:],
                                        scalar2=MGRM_W / (C - 1.0),
                                        op0=OP.mult, op1=OP.mult)
                if do_ea:
                    nc.vector.tensor_copy(res[:, 1:2], lea_ps[:])
                else:
                    nc.gpsimd.memset(res[:, 1:2], 0.0)
                nc.sync.dma_start(out_dr[:, :], res[:])
            else:
                res = wk.tile([1, 2], F32, tag="res")
                if lv == "pool":
                    nc.vector.tensor_copy(res[:], feat64[0:1, 0:2])
                elif lv == "ea":
                    le = wk.tile([1, 1], F32, tag="leadbg")
                    nc.vector.tensor_copy(le[:], lea_ps[:])
                    nc.vector.tensor_scalar(out=res[:, 0:1], in0=le[:],
                                            scalar1=1.0, scalar2=None,
                                            op0=OP.mult)
                    nc.vector.tensor_copy(res[:, 1:2], feat64[0:1, 0:1])
                else:
                    nc.vector.tensor_copy(res[:, 0:1], prT[0:1, 0:1])
                    nc.vector.tensor_copy(res[:, 1:2], feat64[0:1, 0:1])
                nc.sync.dma_start(out_dr[:, :], res[:])

    nc.compile()
    return nc


_NC_CACHE = {}
_last_in_maps = None


def _prep_in_maps(inputs):
    feats = np.asarray(inputs["ins_features"], np.float32)
    logits = np.ascontiguousarray(inputs["class_logits"], dtype=np.float32)
    labels = np.ascontiguousarray(inputs["labels"], dtype=np.int32)
    dom = np.ascontiguousarray(inputs["domain_labels"], dtype=np.int32)
    W1 = np.asarray(inputs["W1"], np.float32)
    W1p = np.ascontiguousarray(
        np.concatenate([W1[:D][PERM], W1[D:]], axis=0).astype(BFnp))
    W2b = np.ascontiguousarray(np.asarray(inputs["W2"], np.float32).astype(BFnp))
    W3b = np.ascontiguousarray(np.asarray(inputs["W3"], np.float32).astype(BFnp))
    Wdb = np.ascontiguousarray(np.asarray(inputs["Wd"], np.float32).astype(BFnp))
    b1 = np.asarray(inputs["b1"], np.float32).astype(BFnp).reshape(1, H1)
    b2 = np.asarray(inputs["b2"], np.float32).astype(BFnp).reshape(1, H1)
    b3 = np.asarray(inputs["b3"], np.float32).astype(BFnp).reshape(1, H1)
    bd = np.asarray(inputs["bd"], np.float32).astype(BFnp).reshape(1, 1)

    # fp8-e4m3 stream; d = 16p+g so feats_8 is [n, 128p, 16g, 49w]
    feats_8 = feats.reshape(N, 128, 16, 49).astype(F8np)
    logits_bf = logits.astype(BFnp)

    # selector for the PE-pooled half: pair q = 50n + w -> sample n
    # (w = 49 is padding; its data rows are zero so E there is harmless)
    qn = np.arange(128)
    Enw = np.zeros((25, 128, NL), F8np)
    for j in range(25):
        Enw[j, qn, (128 * j + qn) // 50] = 1.0
    Enw_flat = np.ascontiguousarray(Enw.transpose(1, 0, 2)).reshape(128, -1)

    in_maps = []
    for k in range(NCORES):
        r0 = NL * k
        sel_s = np.zeros((NS, NL), np.float32)
        sel_t = np.zeros((NS, NL), np.float32)
        if r0 + NL <= NS:
            sel_s[np.arange(r0, r0 + NL), np.arange(NL)] = 1.0
        else:
            sel_t[np.arange(r0 - NS, r0 - NS + NL), np.arange(NL)] = 1.0
        # pair-major [(n, w) padded to 50, f = 128g+p]
        pe = np.zeros((NL, 50, D), F8np)
        pe[:, 0:49, :] = feats_8[r0:r0 + NL].transpose(0, 3, 2, 1) \
            .reshape(NL, 49, D)
        featP = np.ascontiguousarray(
            pe.reshape(25, 128, D).transpose(1, 0, 2)).reshape(128, -1)
        in_maps.append({
            "featP": featP,
            "Enw": Enw_flat,
            "logits_full": logits,
            "logitsT_loc": np.ascontiguousarray(logits_bf[r0:r0 + NL].T),
            "labels_in": labels,
            "dom_shard": np.ascontiguousarray(dom[r0:r0 + NL].reshape(NL, 1)),
            "sel_src": sel_s,
            "sel_tgt": sel_t,
            "W1p": W1p, "W2in": W2b, "W3in": W3b, "Wdin": Wdb,
            "b1in": b1, "b2in": b2, "b3in": b3, "bdin": bd,
        })
    return in_maps


def kernel(**inputs) -> np.ndarray:
    lv = os.environ.get("KLEVEL", "full")
    if _NC_CACHE.get("lv") != lv:
        _NC_CACHE.clear()
        _NC_CACHE["nc"] = _build()
        _NC_CACHE["lv"] = lv
    nc = _NC_CACHE["nc"]
    in_maps = _prep_in_maps(inputs)
    global _last_in_maps
    _last_in_maps = in_maps
    res = bass_utils.run_bass_kernel_spmd(nc, in_maps,
                                          core_ids=list(range(NCORES)))
    out0 = res.results[0]["out_loss"].reshape(2).astype(np.float32)
    if lv in ("full", "nocc"):
        # loss_ea is returned as per-core partial sums (data-parallel
        # gather): combine on the host
        lea = sum(float(r["out_loss"].reshape(2)[1]) for r in res.results)
        return np.array([out0[0], lea / N], np.float32)
    return out0


# revision 75
# speedup vs baseline: 1.0363x; 1.0363x over previous
"""CategoryAwareDAHEAD Trainium2 kernel (8-core SPMD, data-parallel over ROIs).

V2 design
---------
* Host prep: ins_features cast to bf16 and pre-transposed per core to a
  feature-major layout [128p, 16g, 64n, 49w] (feature d = 16p+g, i.e. the
  same PERM as the baseline opT layout).  W1/W2/W3/Wd/biases cast to bf16
  (W1 rows PERM-permuted).  rel-err of bf16 features+weights vs f32
  reference measured at 9.2e-5 (gate is 2e-2).
* Stream: 16 chunks of [128, 64*49] bf16 (~0.8MB each) on the sync queue;
  per chunk: 7x7-mean on the Pool engine into opT f32, a bf16 scaled copy
  opTb (x1/49) for the EA branch, EA layer-1 PSUM accumulation (bf16
  matmuls), and a PE transpose into sample-major feat64 f32.
  opT is deliberately left unscaled (cosines are scale-invariant).
* MGRM phase-1 (softmax stats, threshold chain, accept weights, local
  gathers) only needs logits/labels: issued first, fully hidden under the
  stream.  Chain identical to baseline (th <- max(th,(th+m)/2) monotone
  reformulation).
* Cross-core reduction: ONE bf16 AllReduce of [128, 1024]: cols 0..1007 =
  per-chunk transposed partials [ps|pt|bw] (16 chunks x 63 cols), col 1008
  = loss_ea partial.  Final cosine math is done from the 63x63 Gram blocks
  (cos is computed from unnormalized Grams + diag norms), replicated on
  every core.
* EA layers: bf16 matmuls, LayerNorm fused as mean/E[x2] reductions + one
  Relu activation with AP scale/bias; only {Exp, Ln, Square} activation
  functions are used so a single act table set covers the whole kernel.

KLEVEL env (debug bisection): pool|ea|mgrm|nocc|full
"""

import os
import sys

for _p in ("/opt/trn_rl_repo", "/root/.axon_site/_ro/trn_rl_repo"):
    if _p not in sys.path:
        sys.path.insert(0, _p)

import numpy as np
import ml_dtypes

import concourse.bacc as bacc
import concourse.mybir as mybir
import concourse.tile as tile
from concourse import bass_utils
from concourse.masks import make_identity, make_upper_triangular, make_lower_triangular

F32 = mybir.dt.float32
BF16 = mybir.dt.bfloat16
FP8 = mybir.dt.float8e4
I32 = mybir.dt.int32
AX = mybir.AxisListType
OP = mybir.AluOpType
ACT = mybir.ActivationFunctionType
BFnp = ml_dtypes.bfloat16
F8np = ml_dtypes.float8_e4m3

NCORES = 8
N, NS, C, D = 512, 256, 21, 2048
NL = N // NCORES          # 64 rows per core
H1 = 1024
DIN = D + C               # 2069
S = 32                    # per-class sequence table length (max count is 18)
THR0, MOM, EPS, LN_EPS, MGRM_W = 0.1, 0.5, 1e-8, 1e-5, 1.0

# feature permutation: new index f holds original d = (f % 128) * 16 + f // 128
PERM = (np.arange(D) % 128) * 16 + np.arange(D) // 128


def _build():
    lv = os.environ.get("KLEVEL", "full")
    do_mgrm = lv in ("mgrm", "nocc", "full")
    do_ea = lv in ("ea", "nocc", "full")
    do_final = lv in ("nocc", "full")

    nc = bacc.Bacc("TRN2", target_bir_lowering=False, debug=False,
                   num_devices=NCORES)

    # ---------------- DRAM I/O ----------------
    # pair-major feature stream [128pair x 25 groups, 2048f]: pooling is
    # done on the PE as selector matmuls, keeping the DVE free for the
    # latency-critical MGRM phase-1 chain
    featp_dr = nc.dram_tensor("featP", [128, 25 * D], FP8,
                              kind="ExternalInput")
    enw_dr = nc.dram_tensor("Enw", [128, 25 * NL], FP8, kind="ExternalInput")
    lg_dr = nc.dram_tensor("logits_full", [N, C], F32, kind="ExternalInput")
    lgT_dr = nc.dram_tensor("logitsT_loc", [C, NL], BF16, kind="ExternalInput")
    lab_dr = nc.dram_tensor("labels_in", [NS], I32, kind="ExternalInput")
    dom_dr = nc.dram_tensor("dom_shard", [NL, 1], I32, kind="ExternalInput")
    sels_dr = nc.dram_tensor("sel_src", [NS, NL], F32, kind="ExternalInput")
    selt_dr = nc.dram_tensor("sel_tgt", [NS, NL], F32, kind="ExternalInput")
    w1_dr = nc.dram_tensor("W1p", [DIN, H1], BF16, kind="ExternalInput")
    w2_dr = nc.dram_tensor("W2in", [H1, H1], BF16, kind="ExternalInput")
    w3_dr = nc.dram_tensor("W3in", [H1, H1], BF16, kind="ExternalInput")
    wd_dr = nc.dram_tensor("Wdin", [H1, 1], BF16, kind="ExternalInput")
    b1_dr = nc.dram_tensor("b1in", [1, H1], BF16, kind="ExternalInput")
    b2_dr = nc.dram_tensor("b2in", [1, H1], BF16, kind="ExternalInput")
    b3_dr = nc.dram_tensor("b3in", [1, H1], BF16, kind="ExternalInput")
    bd_dr = nc.dram_tensor("bdin", [1, 1], BF16, kind="ExternalInput")
    out_dr = nc.dram_tensor("out_loss", [1, 2], F32, kind="ExternalOutput")

    featp3 = featp_dr.ap().rearrange("p (j x) -> p j x", j=25)  # x = 2048

    with tile.TileContext(nc) as tc:
        with (
            tc.tile_pool(name="consts", bufs=1) as cst,
            tc.tile_pool(name="insb", bufs=1) as insb,
            tc.tile_pool(name="featp", bufs=4) as featp,
            tc.tile_pool(name="persist", bufs=1) as per,
            tc.tile_pool(name="wpool", bufs=18) as wp,
            tc.tile_pool(name="wpool2", bufs=16) as wp2,
            tc.tile_pool(name="work", bufs=1) as wk,
            tc.tile_pool(name="pps", bufs=3, space="PSUM") as pps,
            tc.tile_pool(name="pph", bufs=1, space="PSUM") as pph,
            tc.tile_pool(name="dram", bufs=1, space="DRAM") as drp,
        ):
            # ---------------- constants ----------------
            id128 = cst.tile([128, 128], F32, tag="id128")
            make_identity(nc, id128[:])
            ut128 = cst.tile([128, 128], F32, tag="ut128")
            make_upper_triangular(nc, ut128[:], val=1.0, diag=True)
            lt128 = cst.tile([128, 128], F32, tag="lt128")
            make_lower_triangular(nc, lt128[:], val=1.0, diag=True)
            ones = cst.tile([128, 128], F32, tag="ones")
            nc.gpsimd.memset(ones[:], 1.0)
            onesbf = cst.tile([1, NL], BF16, tag="onesbf")
            nc.gpsimd.memset(onesbf[:], 1.0)
            iotaS = cst.tile([128, S], F32, tag="iotaS")   # 1..S per partition
            nc.gpsimd.iota(iotaS[:], [[1, S]], base=1, channel_multiplier=0,
                           allow_small_or_imprecise_dtypes=True)
            iota21 = cst.tile([128, C], F32, tag="iota21")  # 0..20
            nc.gpsimd.iota(iota21[:], [[1, C]], base=0, channel_multiplier=0,
                           allow_small_or_imprecise_dtypes=True)
            iota63 = cst.tile([128, 63], F32, tag="iota63")  # col index 0..62
            nc.gpsimd.iota(iota63[:], [[1, 63]], base=0, channel_multiplier=0,
                           allow_small_or_imprecise_dtypes=True)
            pc21 = cst.tile([128, 1], F32, tag="pc21")       # 21 + partition
            nc.gpsimd.iota(pc21[:], [[1, 1]], base=21, channel_multiplier=1,
                           allow_small_or_imprecise_dtypes=True)
            pc42 = cst.tile([128, 1], F32, tag="pc42")       # 42 + partition
            nc.gpsimd.iota(pc42[:], [[1, 1]], base=42, channel_multiplier=1,
                           allow_small_or_imprecise_dtypes=True)


            # ---------------- small input DMAs ----------------
            # sync queue: lg + labels first (phase-1 needs them), then stream
            lg_sb = insb.tile([128, 4 * C], F32, tag="lg")       # [128, 84]
            nc.sync.dma_start(
                lg_sb[:].rearrange("p (c l) -> p c l", c=4),
                lg_dr.ap().rearrange("(c p) l -> p c l", p=128))
            lab_i = insb.tile([128, 2], I32, tag="labi")
            nc.sync.dma_start(lab_i[:], lab_dr.ap().rearrange(
                "(c p) -> p c", p=128))
            lab_f = insb.tile([128, 2], F32, tag="labf")
            nc.vector.tensor_copy(lab_f[:], lab_i[:])

            # ============ MGRM phase 1: logits-only math ============
            eall = None
            counts = None
            if do_mgrm:
                # per-chunk softmax stats (chunks of 128 samples; 0,1=src 2,3=tgt)
                E_ch, OHP_ch, mlOHP_ch = [], [], []
                for ch in range(4):
                    lg_c = lg_sb[:, C * ch:C * (ch + 1)]
                    mx = wk.tile([128, 1], F32, tag=f"mx{ch}")
                    nc.vector.tensor_reduce(mx[:], lg_c, axis=AX.X, op=OP.max)
                    E = wk.tile([128, C], F32, tag=f"E{ch}")
                    nc.vector.tensor_scalar(out=E[:], in0=lg_c, scalar1=mx[:],
                                            scalar2=None, op0=OP.is_equal)
                    negmx = wk.tile([128, 1], F32, tag=f"nmx{ch}")
                    nc.vector.tensor_scalar_mul(negmx[:], mx[:], -1.0)
                    scr = wk.tile([128, C], F32, tag=f"scr{ch}")
                    den = wk.tile([128, 1], F32, tag=f"den{ch}")
                    nc.scalar.activation(scr[:], lg_c, ACT.Exp, bias=negmx[:],
                                         scale=1.0, accum_out=den[:])
                    ml = wk.tile([128, 1], F32, tag=f"ml{ch}")
                    nc.vector.reciprocal(ml[:], den[:])
                    E_ch.append(E)

                    # position of each sample within its domain's class list
                    dom = ch // 2   # 0 = src, 1 = tgt
                    P_ps = pps.tile([128, C], F32, tag="pps")
                    if ch % 2 == 0:
                        nc.tensor.matmul(P_ps[:], ut128[:], E[:], start=True,
                                         stop=True)
                    else:
                        nc.tensor.matmul(P_ps[:], ones[:], E_ch[2 * dom][:],
                                         start=True, stop=False)
                        nc.tensor.matmul(P_ps[:], ut128[:], E[:], start=False,
                                         stop=True)
                    pos = wk.tile([128, 1], F32, tag=f"pos{ch}")
                    posscr = wk.tile([128, C], F32, tag=f"poss{ch}")
                    nc.vector.tensor_tensor(posscr[:], P_ps[:], E[:],
                                            op=OP.mult)
                    nc.vector.tensor_reduce(pos[:], posscr[:], axis=AX.X,
                                            op=OP.add)
                    OHP = wk.tile([128, S], F32, tag=f"OHP{ch}")
                    nc.vector.tensor_scalar(out=OHP[:], in0=iotaS[:],
                                            scalar1=pos[:], scalar2=None,
                                            op0=OP.is_equal)
                    mlOHP = wk.tile([128, S], F32, tag=f"mlO{ch}")
                    nc.vector.tensor_scalar(out=mlOHP[:], in0=iotaS[:],
                                            scalar1=pos[:], scalar2=ml[:],
                                            op0=OP.is_equal, op1=OP.mult)
                    OHP_ch.append(OHP)
                    mlOHP_ch.append(mlOHP)

                sels_sb = insb.tile([128, 2 * NL], F32, tag="sels")
                nc.scalar.dma_start(
                    sels_sb[:].rearrange("p (c n) -> p c n", c=2),
                    sels_dr.ap().rearrange("(c p) n -> p c n", p=128))
                selt_sb = insb.tile([128, 2 * NL], F32, tag="selt")
                nc.scalar.dma_start(
                    selt_sb[:].rearrange("p (c n) -> p c n", c=2),
                    selt_dr.ap().rearrange("(c p) n -> p c n", p=128))

                # tables T[21, 2S]: cols 0..S-1 src, S..2S-1 tgt (dom-major)
                T_all = wk.tile([C, 2 * S], F32, tag="Tall")
                for dom in range(2):
                    T_ps = pps.tile([C, S], F32, tag="pps")
                    nc.tensor.matmul(T_ps[:], E_ch[2 * dom][:],
                                     mlOHP_ch[2 * dom][:], start=True,
                                     stop=False)
                    nc.tensor.matmul(T_ps[:], E_ch[2 * dom + 1][:],
                                     mlOHP_ch[2 * dom + 1][:], start=False,
                                     stop=True)
                    nc.vector.tensor_copy(T_all[:, S * dom:S * (dom + 1)],
                                          T_ps[:])

                # ------------- sequential threshold chain -------------
                # th history keeps the serial loop at 2 DVE ops/step; all 32
                # accept masks come from ONE vectorized is_ge afterwards
                T3 = T_all[:].rearrange("p (d s) -> p d s", d=2)
                thh = wk.tile([C, 2 * (S + 1)], F32, tag="thh")
                th3 = thh[:].rearrange("p (s d) -> p s d", s=S + 1)
                nc.gpsimd.memset(th3[:, 0, :], THR0)
                tmp = wk.tile([C, 2], F32, tag="chtmp")
                for s in range(S):
                    nc.vector.tensor_tensor(tmp[:], T3[:, :, s], th3[:, s, :],
                                            op=OP.add)
                    nc.vector.scalar_tensor_tensor(
                        out=th3[:, s + 1, :], in0=tmp[:], scalar=0.5,
                        in1=th3[:, s, :], op0=OP.mult, op1=OP.max)
                A = wk.tile([C, 2 * S], F32, tag="A")
                A3 = A[:].rearrange("p (d s) -> p d s", d=2)
                nc.vector.tensor_tensor(
                    A3[:].rearrange("p d s -> p s d"), T3[:].rearrange(
                        "p d s -> p s d"), th3[:, 0:S, :], op=OP.is_ge)

                # per-domain accept-index math -> wtab [21, 2S]
                # w_j = prod_{i>=j, accepted} (1/J_i): gated reverse product
                # scan (no Ln/Exp -> no act-table switches)
                gt = wk.tile([C, 2 * S], F32, tag="gt")
                gt3 = gt[:].rearrange("p (d s) -> p d s", d=2)
                for dom in range(2):
                    A_dom = A[:, S * dom:S * (dom + 1)]
                    A_T = wk.tile([S, C], F32, tag=f"AT{dom}")
                    at_ps = pps.tile([S, C], F32, tag="pps")
                    nc.tensor.transpose(at_ps[:], A_dom, id128[0:C, 0:C])
                    nc.vector.tensor_copy(A_T[:], at_ps[:])
                    J_ps = pps.tile([C, S], F32, tag="pps")
                    nc.tensor.matmul(J_ps[:], A_T[:], ut128[0:S, 0:S],
                                     start=True, stop=True)
                    jc = wk.tile([C, S], F32, tag=f"jc{dom}")
                    nc.vector.tensor_scalar(out=jc[:], in0=J_ps[:], scalar1=1.0,
                                            scalar2=None, op0=OP.max)
                    rj = wk.tile([C, S], F32, tag=f"rj{dom}")
                    nc.vector.reciprocal(rj[:], jc[:])
                    # g = A ? 1/J : 1  =  (1/J)*A + (1 - A)
                    am = wk.tile([C, S], F32, tag=f"am{dom}")
                    nc.vector.tensor_scalar(out=am[:], in0=A_dom, scalar1=-1.0,
                                            scalar2=1.0, op0=OP.mult,
                                            op1=OP.add)
                    gd = gt3[:, dom, :]
                    nc.vector.tensor_tensor(gd, rj[:], A_dom, op=OP.mult)
                    nc.vector.tensor_tensor(gd, gd, am[:], op=OP.add)
                # suffix product scan (Pool engine), then wtab = pw * A
                pw = wk.tile([C, 2 * S], F32, tag="pw")
                pw3 = pw[:].rearrange("p (d s) -> p d s", d=2)
                nc.vector.tensor_copy(pw3[:, :, S - 1], gt3[:, :, S - 1])
                for s in range(S - 2, -1, -1):
                    nc.vector.tensor_tensor(pw3[:, :, s], gt3[:, :, s],
                                            pw3[:, :, s + 1], op=OP.mult)
                wtab = wk.tile([C, 2 * S], F32, tag="wtab")
                nc.vector.tensor_tensor(wtab[:], pw[:], A[:], op=OP.mult)

                # ---------- per-sample weights, local gathers ----------
                # eall [64, 63+1]: cols 0:21 src EW, 21:42 tgt EW, 42:63 label
                eall = wk.tile([NL, 64], BF16, tag="eall")
                for dom in range(2):
                    sel = sels_sb if dom == 0 else selt_sb
                    ewl_ps = pps.tile([NL, C], F32, tag="pps")
                    for cc in range(2):
                        ch = 2 * dom + cc
                        ET = wk.tile([C, 128], F32, tag=f"ET{ch}")
                        et_ps = pps.tile([C, 128], F32, tag="pps")
                        nc.tensor.transpose(et_ps[:], E_ch[ch][:], id128[:])
                        nc.vector.tensor_copy(ET[:], et_ps[:])
                        G_ps = pps.tile([128, S], F32, tag="pps")
                        nc.tensor.matmul(G_ps[:], ET[:],
                                         wtab[:, S * dom:S * (dom + 1)],
                                         start=True, stop=True)
                        ws = wk.tile([128, 1], F32, tag=f"ws{ch}")
                        wscr = wk.tile([128, S], F32, tag=f"wscr{ch}")
                        nc.vector.tensor_tensor(wscr[:], G_ps[:],
                                                OHP_ch[ch][:], op=OP.mult)
                        nc.vector.tensor_reduce(ws[:], wscr[:], axis=AX.X,
                                                op=OP.add)
                        EW = wk.tile([128, C], F32, tag=f"EW{ch}")
                        nc.vector.tensor_scalar(out=EW[:], in0=E_ch[ch][:],
                                                scalar1=ws[:], scalar2=None,
                                                op0=OP.mult)
                        nc.tensor.matmul(ewl_ps[:],
                                         sel[:, NL * cc:NL * (cc + 1)],
                                         EW[:], start=(cc == 0), stop=(cc == 1))
                    nc.vector.tensor_copy(eall[:, C * dom:C * (dom + 1)],
                                          ewl_ps[:])

                # labels onehot + counts + local label gather
                elab_l_ps = pps.tile([NL, C], F32, tag="pps")
                cnt_ps = pps.tile([C, 1], F32, tag="pps")
                for cc in range(2):
                    Elab = wk.tile([128, C], F32, tag=f"Elab{cc}")
                    nc.vector.tensor_scalar(out=Elab[:], in0=iota21[:],
                                            scalar1=lab_f[:, cc:cc + 1],
                                            scalar2=None, op0=OP.is_equal)
                    nc.tensor.matmul(cnt_ps[:], Elab[:], ones[:, 0:1],
                                     start=(cc == 0), stop=(cc == 1))
                    nc.tensor.matmul(elab_l_ps[:],
                                     sels_sb[:, NL * cc:NL * (cc + 1)],
                                     Elab[:], start=(cc == 0), stop=(cc == 1))
                nc.vector.tensor_copy(eall[:, 2 * C:3 * C], elab_l_ps[:])
                counts = wk.tile([C, 1], F32, tag="counts")
                nc.vector.tensor_copy(counts[:], cnt_ps[:])

            # ============ feature stream + pool + EA-L1 + feat64 ============
            opTb = per.tile([128, 16 * NL], BF16, tag="opTb")  # x(1/49), bf16
            opTb3 = opTb[:].rearrange("p (g n) -> p g n", g=16)
            feat64 = per.tile([NL, D], BF16, tag="feat64")     # sample-major

            bounce = drp.tile([128, 1008], FP8, tag="bounce")
            prT = per.tile([128, 16 * 63], FP8, tag="prT")
            id128b = cst.tile([128, 128], BF16, tag="id128b")
            nc.vector.tensor_copy(id128b[:], id128[:])

            def prt_chunks(gs):
                for g in gs:
                    pr_ps = pps.tile([128, 63], F32, tag="pps")
                    nc.tensor.matmul(pr_ps[:], feat64[:, 128 * g:128 * (g + 1)],
                                     eall[:, 0:63], start=True, stop=True)
                    nc.vector.tensor_copy(prT[:, 63 * g:63 * (g + 1)], pr_ps[:])

            enw_sb = insb.tile([128, 25 * NL], FP8, tag="enw")
            nc.scalar.dma_start(enw_sb[:], enw_dr[:, :])
            lgT_sb = insb.tile([C, NL], BF16, tag="lgT")
            nc.scalar.dma_start(lgT_sb[:], lgT_dr[:, :])
            b1_sb = insb.tile([1, H1], BF16, tag="b1")
            nc.scalar.dma_start(b1_sb[:], b1_dr[:, :])
            b2_sb = insb.tile([1, H1], BF16, tag="b2")
            nc.scalar.dma_start(b2_sb[:], b2_dr[:, :])
            b3_sb = insb.tile([1, H1], BF16, tag="b3")
            nc.scalar.dma_start(b3_sb[:], b3_dr[:, :])
            bd_sb = insb.tile([1, 1], BF16, tag="bd")
            nc.scalar.dma_start(bd_sb[:], bd_dr[:, :])
            wd_sb = insb.tile([128, 8], BF16, tag="wd")
            nc.scalar.dma_start(
                wd_sb[:].rearrange("p (c o) -> p c o", c=8),
                wd_dr.ap().rearrange("(c p) o -> p c o", p=128))
            dom_i = insb.tile([NL, 1], I32, tag="domi")
            nc.scalar.dma_start(dom_i[:], dom_dr[:, :])
            dom_f = insb.tile([NL, 1], F32, tag="domf")
            nc.vector.tensor_copy(dom_f[:], dom_i[:])

            # --- stream: 25 pair-major chunks, pooled on the PE ---
            # pool_ps accumulates the full [64n, 2048f] pooled sums (4 psum
            # banks); L1 runs post-stream so its 2 banks never coexist with
            # a partial pool.
            pool_ps = pph.tile([NL, D], F32, tag="hbig")
            # 12 double-group chunks via fp8 DoubleRow matmuls (0.5 cyc/row:
            # out += E_j^T @ X_j + E_j+1^T @ X_j+1 in one instruction),
            # plus a final single group
            MM2 = mybir.MatmulPerfMode.DoubleRow
            for jp in range(12):
                ftp = featp.tile([128, 2 * D], FP8, tag="ftp")
                nc.sync.dma_start(
                    ftp[:].rearrange("p (g x) -> p g x", g=2),
                    featp3[:, 2 * jp:2 * jp + 2, :])
                ftp3 = ftp[:].rearrange("p (g x) -> p g x", g=2)
                lhs3 = enw_sb[:, 2 * NL * jp:2 * NL * (jp + 1)].rearrange(
                    "p (g n) -> p g n", g=2)
                for q in range(4):
                    nc.tensor.matmul(pool_ps[:, 512 * q:512 * (q + 1)],
                                     lhs3,
                                     ftp3[:, :, 512 * q:512 * (q + 1)],
                                     start=(jp == 0), stop=False,
                                     perf_mode=MM2)
            ftp = featp.tile([128, 2 * D], FP8, tag="ftp")
            nc.sync.dma_start(ftp[0:128, 0:D], featp3[:, 24, :])
            for q in range(4):
                nsl = slice(512 * q, 512 * (q + 1))
                nc.tensor.matmul(pool_ps[:, nsl],
                                 enw_sb[:, NL * 24:NL * 25],
                                 ftp[0:128, nsl],
                                 start=False, stop=True)

            # W1 + W2/W3 prefetch (act queue drains these mid-stream)
            w1ts = {}
            if do_ea:
                for g in range(16):
                    w1t = wp.tile([128, H1], BF16, tag="w")
                    w1ts[g] = w1t
                    nc.scalar.dma_start(w1t[:],
                                        w1_dr[128 * g:128 * (g + 1), :])
                wt16 = wp.tile([128, H1], BF16, tag="w")
                nc.scalar.dma_start(wt16[0:C, :], w1_dr[D:D + C, :])
                wpre = {}
                for wi, w_dr in enumerate((w2_dr, w3_dr)):
                    for kc in range(8):
                        wpt = wp2.tile([128, H1], BF16, tag="w2")
                        wpre[(wi, kc)] = wpt
                        # the fp8 stream is short: both W2 and W3 ride the
                        # sync queue after it, keeping the act queue clear
                        # for the post-pool copies
                        nc.sync.dma_start(wpt[:],
                                          w_dr[128 * kc:128 * (kc + 1), :])

            # pooled sums -> feat64 (sample-major, bf16) per psum quadrant,
            # with the AR-gating prototype partials pipelined right behind
            for q in range(4):
                nsl = slice(512 * q, 512 * (q + 1))
                if q % 2 == 0:
                    nc.scalar.activation(feat64[:, nsl], pool_ps[:, nsl],
                                         ACT.Copy)
                else:
                    nc.vector.tensor_copy(feat64[:, nsl], pool_ps[:, nsl])
                if do_mgrm:
                    for g2 in range(4 * q, 4 * q + 4, 2):
                        pr2_ps = pps.tile([128, 126], F32, tag="pps")
                        for k in range(2):
                            g = g2 + k
                            nc.tensor.matmul(pr2_ps[:, 63 * k:63 * (k + 1)],
                                             feat64[:, 128 * g:128 * (g + 1)],
                                             eall[:, 0:63], start=True,
                                             stop=True)
                        if (g2 // 2) % 2 == 0:
                            nc.vector.tensor_copy(
                                prT[:, 63 * g2:63 * (g2 + 2)], pr2_ps[:])
                        else:
                            nc.scalar.activation(
                                prT[:, 63 * g2:63 * (g2 + 2)], pr2_ps[:],
                                ACT.Copy)
            if do_mgrm:
                # SWDGE on the Pool queue: the completion sem is local to the
                # engine that fires the ReduceScatter right after
                nc.gpsimd.dma_start(bounce[:], prT[:])
            # opT-orientation bf16 copies for the EA layer-1 lhsT
            for g in range(16):
                tp2_ps = pps.tile([128, NL], BF16, tag="pps")
                nc.tensor.transpose(tp2_ps[:],
                                    feat64[:, 128 * g:128 * (g + 1)],
                                    id128b[0:NL, 0:NL])
                nc.scalar.activation(opTb3[:, g, :], tp2_ps[:], ACT.Copy,
                                     scale=1.0 / 49.0)
            if do_ea:
                h_big = pph.tile([NL, D], F32, tag="hbig")
                h_ps = h_big[:, 0:H1]
                for half in range(2):
                    nsl = slice(512 * half, 512 * (half + 1))
                    nc.tensor.matmul(h_ps[:, nsl], onesbf[:], b1_sb[:, nsl],
                                     start=True, stop=False)
                for half in range(2):
                    nsl = slice(512 * half, 512 * (half + 1))
                    nc.tensor.matmul(h_ps[:, nsl], lgT_sb[:], wt16[0:C, nsl],
                                     start=False, stop=False)
                for g in range(16):
                    for half in range(2):
                        nsl = slice(512 * half, 512 * (half + 1))
                        nc.tensor.matmul(h_ps[:, nsl], opTb3[:, g, :],
                                         w1ts[g][:, nsl], start=False,
                                         stop=(g == 15))

            # ============ EA layers ============
            U16 = mybir.dt.uint16

            def rsqrt16(y, x, shape, tg):
                """y = x^-0.5 on DVE: bf16 bit-trick seed + 2 Newton steps.
                (no rsqrt opcode on DVE; act-table Sqrt would thrash the
                single {exp,ln,square} table set.  The DVE ALU datapath is
                fp32 internally so only 16-bit ints survive bit tricks.)"""
                xb = wk.tile(shape, BF16, tag=f"rsqb{tg}")
                nc.vector.tensor_copy(xb[:], x[:])
                t16 = wk.tile(shape, U16, tag=f"rsqs{tg}")
                nc.vector.tensor_scalar(out=t16[:], in0=xb[:].bitcast(U16),
                                        scalar1=1, scalar2=None,
                                        op0=OP.logical_shift_right)
                y16 = wk.tile(shape, U16, tag=f"rsqy{tg}")
                nc.vector.tensor_scalar(out=y16[:], in0=t16[:], scalar1=-1.0,
                                        scalar2=float(0x5F37), op0=OP.mult,
                                        op1=OP.add)
                t1 = wk.tile(shape, F32, tag=f"rsqt{tg}")
                nc.vector.tensor_copy(y[:], y16[:].bitcast(BF16))
                for _ in range(2):
                    nc.vector.tensor_tensor(t1[:], y[:], y[:], op=OP.mult)
                    nc.vector.tensor_tensor(t1[:], t1[:], x[:], op=OP.mult)
                    nc.vector.tensor_scalar(out=t1[:], in0=t1[:], scalar1=-0.5,
                                            scalar2=1.5, op0=OP.mult,
                                            op1=OP.add)
                    nc.vector.tensor_tensor(y[:], y[:], t1[:], op=OP.mult)

            def ln_relu(h_in, scaled):
                """relu(layernorm(h_in)) -> f32 [NL, H1].

                scaled=False drops the 1/std factor: relu(c*x) = c*relu(x)
                and the NEXT LayerNorm removes any per-row scale exactly
                (biases are zero in this problem), so only the last LN
                before the sigmoid needs the true rstd."""
                musum = wk.tile([NL, 1], F32, tag="mu")
                nc.vector.tensor_reduce(musum[:], h_in[:], axis=AX.X,
                                        op=OP.add)
                mu = wk.tile([NL, 1], F32, tag="mus")
                nc.vector.tensor_scalar_mul(mu[:], musum[:], 1.0 / H1)
                h = wk.tile([NL, H1], F32, tag="h")
                if not scaled:
                    nmu = wk.tile([NL, 1], F32, tag="nmu")
                    nc.vector.tensor_scalar_mul(nmu[:], musum[:], -1.0 / H1)
                    nc.scalar.activation(h[:], h_in[:], ACT.Relu, bias=nmu[:])
                    return h
                sqs = wk.tile([NL, H1], F32, tag="sqs")
                q = wk.tile([NL, 1], F32, tag="q")
                nc.scalar.activation(sqs[:], h_in[:], ACT.Square,
                                     accum_out=q[:])
                nmu2e = wk.tile([NL, 1], F32, tag="nmu2e")
                nc.vector.tensor_scalar(out=nmu2e[:], in0=mu[:], scalar1=mu[:],
                                        scalar2=-1.0, op0=OP.mult, op1=OP.mult)
                nc.vector.tensor_scalar(out=nmu2e[:], in0=nmu2e[:],
                                        scalar1=float(LN_EPS), scalar2=None,
                                        op0=OP.add)
                ve = wk.tile([NL, 1], F32, tag="ve")
                nc.vector.tensor_scalar(out=ve[:], in0=q[:], scalar1=1.0 / H1,
                                        scalar2=nmu2e[:], op0=OP.mult,
                                        op1=OP.add)
                rstd = wk.tile([NL, 1], F32, tag="rstd")
                rsqrt16(rstd, ve, [NL, 1], "ln")
                nb = wk.tile([NL, 1], F32, tag="nb")
                nc.vector.tensor_scalar(out=nb[:], in0=mu[:], scalar1=rstd[:],
                                        scalar2=-1.0, op0=OP.mult, op1=OP.mult)
                nc.scalar.activation(h[:], h_in[:], ACT.Relu, scale=rstd[:],
                                     bias=nb[:])
                return h

            def transpose_h(h):
                """[NL, H1] f32 -> [128, 8*NL] bf16 (chunk j = features 128j..)"""
                hT = wk.tile([128, 8 * NL], BF16, tag="hT")
                for half in range(2):
                    ht_ps = pps.tile([128, 4 * NL], F32, tag="pps")
                    for j in range(4):
                        jj = 4 * half + j
                        nc.tensor.transpose(ht_ps[:, NL * j:NL * (j + 1)],
                                            h[:, 128 * jj:128 * (jj + 1)],
                                            id128[0:NL, 0:NL])
                    nc.vector.tensor_copy(
                        hT[:, 4 * NL * half:4 * NL * (half + 1)], ht_ps[:])
                return hT

            lea_ps = None
            if do_ea:
                h = ln_relu(h_ps[:], scaled=True)
                for li_w, (w_dr, b_sb) in enumerate(((w2_dr, b2_sb),
                                                     (w3_dr, b3_sb))):
                    hT = transpose_h(h)
                    h_big = pph.tile([NL, D], F32, tag="hbig")
                    h_ps = h_big[:, 0:H1]
                    for half in range(2):
                        nsl = slice(512 * half, 512 * (half + 1))
                        nc.tensor.matmul(h_ps[:, nsl], onesbf[:], b_sb[:, nsl],
                                         start=True, stop=False)
                    for kc in range(8):
                        wt = wpre[(li_w, kc)]
                        for half in range(2):
                            nsl = slice(512 * half, 512 * (half + 1))
                            nc.tensor.matmul(h_ps[:, nsl],
                                             hT[:, NL * kc:NL * (kc + 1)],
                                             wt[:, nsl],
                                             start=False,
                                             stop=(kc == 7))
                    h = ln_relu(h_ps[:], scaled=True)

                h3T = transpose_h(h)
                zd_ps = pps.tile([NL, 1], F32, tag="pps")
                nc.tensor.matmul(zd_ps[:], onesbf[:], bd_sb[:],
                                 start=True, stop=False)
                for kc in range(8):
                    nc.tensor.matmul(zd_ps[:], h3T[:, NL * kc:NL * (kc + 1)],
                                     wd_sb[:, kc:kc + 1], start=False,
                                     stop=(kc == 7))
                # z = sigmoid(zd) = 1/(1+exp(-zd))
                enz = wk.tile([NL, 1], F32, tag="enz")
                nc.scalar.activation(enz[:], zd_ps[:], ACT.Exp, scale=-1.0)
                zden = wk.tile([NL, 1], F32, tag="zden")
                nc.vector.tensor_scalar(out=zden[:], in0=enz[:], scalar1=1.0,
                                        scalar2=None, op0=OP.add)
                z = wk.tile([NL, 1], F32, tag="z")
                nc.vector.reciprocal(z[:], zden[:])
                # softplus(-z) = ln(1 + exp(-z))
                emz = wk.tile([NL, 1], F32, tag="emz")
                nc.scalar.activation(emz[:], z[:], ACT.Exp, scale=-1.0)
                sp = wk.tile([NL, 1], F32, tag="sp")
                nc.scalar.activation(sp[:], emz[:], ACT.Ln,
                                     bias=ones[0:NL, 0:1])
                omy = wk.tile([NL, 1], F32, tag="omy")
                nc.vector.tensor_scalar(out=omy[:], in0=dom_f[:], scalar1=-1.0,
                                        scalar2=1.0, op0=OP.mult, op1=OP.add)
                li_t = wk.tile([NL, 1], F32, tag="li")
                nc.vector.scalar_tensor_tensor(out=li_t[:], in0=z[:],
                                               scalar=omy[:], in1=sp[:],
                                               op0=OP.mult, op1=OP.add)
                # per-core loss_ea partial: reduced on the HOST (data-parallel
                # gather), so the AllReduce is not gated on the EA branch
                lea_ps = pps.tile([1, 1], F32, tag="pps")
                nc.tensor.matmul(lea_ps[:], li_t[:], ones[0:NL, 0:1],
                                 start=True, stop=True)


            if do_final:
                # ---- cross-core reduce: ReduceScatter + AllGather ----
                # (cheaper than AllReduce: no 1.875x single-instr penalty)
                bounce_out = drp.tile([128, 1008], FP8, tag="bounce_out",
                                      addr_space="Shared")
                if lv == "nocc":
                    nc.sync.dma_start(bounce_out[:], bounce[:])
                else:
                    bmid = drp.tile([16, 1008], FP8, tag="bmid")
                    nc.gpsimd.collective_compute(
                        "ReduceScatter", OP.add,
                        replica_groups=[list(range(NCORES))],
                        ins=[bounce[:].opt()], outs=[bmid[:].opt()])
                    nc.gpsimd.collective_compute(
                        "AllGather", OP.bypass,
                        replica_groups=[list(range(NCORES))],
                        ins=[bmid[:].opt()], outs=[bounce_out[:].opt()])
                XT = per.tile([128, 16 * 63], FP8, tag="XT")
                nc.sync.dma_start(XT[:], bounce_out[:])

                # ---------------- final (replicated) ----------------
                # full Gram G[63,63] of the stacked [ps|pt|bw] prototypes;
                # diag = squared norms, blocks = cross inner products
                G_ps = pps.tile([63, 63], F32, tag="pps")
                for g in range(16):
                    base = 63 * g
                    nc.tensor.matmul(G_ps[:], XT[:, base:base + 63],
                                     XT[:, base:base + 63],
                                     start=(g == 0), stop=(g == 15))
                G_sb = wk.tile([63, 63], F32, tag="Gsb")
                nc.vector.tensor_copy(G_sb[:], G_ps[:])
                # realign pt rows (21..41) and bw rows (42..62) down to
                # partitions 0..20 with shifted-identity matmuls
                Gpt_ps = pps.tile([C, 63], F32, tag="pps")
                nc.tensor.matmul(Gpt_ps[:], id128[0:63, C:2 * C], G_sb[:],
                                 start=True, stop=True)
                Gbw_ps = pps.tile([C, 63], F32, tag="pps")
                nc.tensor.matmul(Gbw_ps[:], id128[0:63, 2 * C:3 * C], G_sb[:],
                                 start=True, stop=True)
                # diag extraction masks on partitions 0..20
                mpt = wk.tile([C, 63], F32, tag="mpt")
                nc.vector.tensor_scalar(out=mpt[:], in0=iota63[0:C, :],
                                        scalar1=pc21[0:C, :], scalar2=None,
                                        op0=OP.is_equal)
                mbw = wk.tile([C, 63], F32, tag="mbw")
                nc.vector.tensor_scalar(out=mbw[:], in0=iota63[0:C, :],
                                        scalar1=pc42[0:C, :], scalar2=None,
                                        op0=OP.is_equal)
                # d3 cols: 0 = ps, 1 = pt, 2 = bw squared norms
                d3 = wk.tile([C, 3], F32, tag="d3")
                dsc = wk.tile([C, 63], F32, tag="dsc")
                nc.vector.tensor_tensor(dsc[:], G_sb[0:C, :], id128[0:C, 0:63],
                                        op=OP.mult)
                nc.vector.tensor_reduce(d3[:, 0:1], dsc[:], axis=AX.X,
                                        op=OP.add)
                nc.vector.tensor_tensor(dsc[:], Gpt_ps[:], mpt[:], op=OP.mult)
                nc.vector.tensor_reduce(d3[:, 1:2], dsc[:], axis=AX.X,
                                        op=OP.add)
                nc.vector.tensor_tensor(dsc[:], Gbw_ps[:], mbw[:], op=OP.mult)
                nc.vector.tensor_reduce(d3[:, 2:3], dsc[:], axis=AX.X,
                                        op=OP.add)
                # rn3 = 1/max(sqrt(d), eps) = max(d, eps^2)^-0.5
                dmx3 = wk.tile([C, 3], F32, tag="dmx3")
                nc.vector.tensor_scalar(out=dmx3[:], in0=d3[:],
                                        scalar1=float(EPS) ** 2, scalar2=None,
                                        op0=OP.max)
                rn3 = wk.tile([C, 3], F32, tag="rn3")
                rsqrt16(rn3, dmx3, [C, 3], "rn")

                cosP = wk.tile([C, C], F32, tag="cosP")
                nc.vector.tensor_scalar(out=cosP[:],
                                        in0=G_sb[0:C, C:2 * C],
                                        scalar1=rn3[:, 0:1], scalar2=None,
                                        op0=OP.mult)
                cosB = wk.tile([C, C], F32, tag="cosB")
                nc.vector.tensor_scalar(out=cosB[:], in0=Gbw_ps[:, C:2 * C],
                                        scalar1=rn3[:, 2:3], scalar2=None,
                                        op0=OP.mult)
                absd = wk.tile([C, C], F32, tag="absd")
                nc.vector.tensor_tensor(absd[:], cosB[:], cosP[:],
                                        op=OP.subtract)
                nc.scalar.activation(absd[:], absd[:], ACT.Abs)
                # cem = (counts > 0) with class 0 zeroed; wv = rn_t * cem
                cem = wk.tile([C, 1], F32, tag="cem")
                nc.vector.tensor_scalar(out=cem[:], in0=counts[:], scalar1=0.0,
                                        scalar2=None, op0=OP.is_gt)
                nc.gpsimd.memset(cem[0:1, :], 0.0)
                wv = wk.tile([C, 1], F32, tag="wv")
                nc.vector.tensor_tensor(wv[:], rn3[:, 1:2], cem[:],
                                        op=OP.mult)
                # sum over j = 1..20 only (reference drops row/col 0)
                nc.gpsimd.memset(absd[0:1, :], 0.0)
                s1_ps = pps.tile([C, 1], F32, tag="pps")
                nc.tensor.matmul(s1_ps[:], absd[:], ones[0:C, 0:1],
                                 start=True, stop=True)
                s1 = wk.tile([C, 1], F32, tag="s1")
                nc.vector.tensor_copy(s1[:], s1_ps[:])
                tot_ps = pps.tile([1, 1], F32, tag="pps")
                nc.tensor.matmul(tot_ps[:], s1[:], wv[:], start=True, stop=True)
                nm_ps = pps.tile([1, 1], F32, tag="pps")
                nc.tensor.matmul(nm_ps[:], cem[:], ones[0:C, 0:1],
                                 start=True, stop=True)
                nm_sb = wk.tile([1, 1], F32, tag="nmsb")
                nc.vector.tensor_copy(nm_sb[:], nm_ps[:])
                rnm = wk.tile([1, 1], F32, tag="rnm")
                nc.vector.reciprocal(rnm[:], nm_sb[:])

                res = wk.tile([1, 2], F32, tag="res")
                nc.vector.tensor_scalar(out=res[:, 0:1], in0=tot_ps[:],
                                        scalar1=rnm[:],
                                        scalar2=MGRM_W / (C - 1.0),
                                        op0=OP.mult, op1=OP.mult)
                if do_ea:
                    nc.vector.tensor_copy(res[:, 1:2], lea_ps[:])
                else:
                    nc.gpsimd.memset(res[:, 1:2], 0.0)
                nc.sync.dma_start(out_dr[:, :], res[:])
            else:
                res = wk.tile([1, 2], F32, tag="res")
                if lv == "pool":
                    nc.vector.tensor_copy(res[:], feat64[0:1, 0:2])
                elif lv == "ea":
                    le = wk.tile([1, 1], F32, tag="leadbg")
                    nc.vector.tensor_copy(le[:], lea_ps[:])
                    nc.vector.tensor_scalar(out=res[:, 0:1], in0=le[:],
                                            scalar1=1.0, scalar2=None,
                                            op0=OP.mult)
                    nc.vector.tensor_copy(res[:, 1:2], feat64[0:1, 0:1])
                else:
                    nc.vector.tensor_copy(res[:, 0:1], prT[0:1, 0:1])
                    nc.vector.tensor_copy(res[:, 1:2], feat64[0:1, 0:1])
                nc.sync.dma_start(out_dr[:, :], res[:])

    nc.compile()
    return nc


_NC_CACHE = {}
_last_in_maps = None


def _prep_in_maps(inputs):
    feats = np.asarray(inputs["ins_features"], np.float32)
    logits = np.ascontiguousarray(inputs["class_logits"], dtype=np.float32)
    labels = np.ascontiguousarray(inputs["labels"], dtype=np.int32)
    dom = np.ascontiguousarray(inputs["domain_labels"], dtype=np.int32)
    W1 = np.asarray(inputs["W1"], np.float32)
    W1p = np.ascontiguousarray(
        np.concatenate([W1[:D][PERM], W1[D:]], axis=0).astype(BFnp))
    W2b = np.ascontiguousarray(np.asarray(inputs["W2"], np.float32).astype(BFnp))
    W3b = np.ascontiguousarray(np.asarray(inputs["W3"], np.float32).astype(BFnp))
    Wdb = np.ascontiguousarray(np.asarray(inputs["Wd"], np.float32).astype(BFnp))
    b1 = np.asarray(inputs["b1"], np.float32).astype(BFnp).reshape(1, H1)
    b2 = np.asarray(inputs["b2"], np.float32).astype(BFnp).reshape(1, H1)
    b3 = np.asarray(inputs["b3"], np.float32).astype(BFnp).reshape(1, H1)
    bd = np.asarray(inputs["bd"], np.float32).astype(BFnp).reshape(1, 1)

    # fp8-e4m3 stream; d = 16p+g so feats_8 is [n, 128p, 16g, 49w]
    feats_8 = feats.reshape(N, 128, 16, 49).astype(F8np)
    logits_bf = logits.astype(BFnp)

    # selector for the PE-pooled half: pair q = 50n + w -> sample n
    # (w = 49 is padding; its data rows are zero so E there is harmless)
    qn = np.arange(128)
    Enw = np.zeros((25, 128, NL), F8np)
    for j in range(25):
        Enw[j, qn, (128 * j + qn) // 50] = 1.0
    Enw_flat = np.ascontiguousarray(Enw.transpose(1, 0, 2)).reshape(128, -1)

    in_maps = []
    for k in range(NCORES):
        r0 = NL * k
        sel_s = np.zeros((NS, NL), np.float32)
        sel_t = np.zeros((NS, NL), np.float32)
        if r0 + NL <= NS:
            sel_s[np.arange(r0, r0 + NL), np.arange(NL)] = 1.0
        else:
            sel_t[np.arange(r0 - NS, r0 - NS + NL), np.arange(NL)] = 1.0
        # pair-major [(n, w) padded to 50, f = 128g+p]
        pe = np.zeros((NL, 50, D), F8np)
        pe[:, 0:49, :] = feats_8[r0:r0 + NL].transpose(0, 3, 2, 1) \
            .reshape(NL, 49, D)
        featP = np.ascontiguousarray(
            pe.reshape(25, 128, D).transpose(1, 0, 2)).reshape(128, -1)
        in_maps.append({
            "featP": featP,
            "Enw": Enw_flat,
            "logits_full": logits,
            "logitsT_loc": np.ascontiguousarray(logits_bf[r0:r0 + NL].T),
            "labels_in": labels,
            "dom_shard": np.ascontiguousarray(dom[r0:r0 + NL].reshape(NL, 1)),
            "sel_src": sel_s,
            "sel_tgt": sel_t,
            "W1p": W1p, "W2in": W2b, "W3in": W3b, "Wdin": Wdb,
            "b1in": b1, "b2in": b2, "b3in": b3, "bdin": bd,
        })
    return in_maps


def kernel(**inputs) -> np.ndarray:
    lv = os.environ.get("KLEVEL", "full")
    if _NC_CACHE.get("lv") != lv:
        _NC_CACHE.clear()
        _NC_CACHE["nc"] = _build()
        _NC_CACHE["lv"] = lv
    nc = _NC_CACHE["nc"]
    in_maps = _prep_in_maps(inputs)
    global _last_in_maps
    _last_in_maps = in_maps
    res = bass_utils.run_bass_kernel_spmd(nc, in_maps,
                                          core_ids=list(range(NCORES)))
    out0 = res.results[0]["out_loss"].reshape(2).astype(np.float32)
    if lv in ("full", "nocc"):
        # loss_ea is returned as per-core partial sums (data-parallel
        # gather): combine on the host
        lea = sum(float(r["out_loss"].reshape(2)[1]) for r in res.results)
        return np.array([out0[0], lea / N], np.float32)
    return out0


# revision 76
# speedup vs baseline: 1.0477x; 1.0110x over previous
"""CategoryAwareDAHEAD Trainium2 kernel (8-core SPMD, data-parallel over ROIs).

V2 design
---------
* Host prep: ins_features cast to bf16 and pre-transposed per core to a
  feature-major layout [128p, 16g, 64n, 49w] (feature d = 16p+g, i.e. the
  same PERM as the baseline opT layout).  W1/W2/W3/Wd/biases cast to bf16
  (W1 rows PERM-permuted).  rel-err of bf16 features+weights vs f32
  reference measured at 9.2e-5 (gate is 2e-2).
* Stream: 16 chunks of [128, 64*49] bf16 (~0.8MB each) on the sync queue;
  per chunk: 7x7-mean on the Pool engine into opT f32, a bf16 scaled copy
  opTb (x1/49) for the EA branch, EA layer-1 PSUM accumulation (bf16
  matmuls), and a PE transpose into sample-major feat64 f32.
  opT is deliberately left unscaled (cosines are scale-invariant).
* MGRM phase-1 (softmax stats, threshold chain, accept weights, local
  gathers) only needs logits/labels: issued first, fully hidden under the
  stream.  Chain identical to baseline (th <- max(th,(th+m)/2) monotone
  reformulation).
* Cross-core reduction: ONE bf16 AllReduce of [128, 1024]: cols 0..1007 =
  per-chunk transposed partials [ps|pt|bw] (16 chunks x 63 cols), col 1008
  = loss_ea partial.  Final cosine math is done from the 63x63 Gram blocks
  (cos is computed from unnormalized Grams + diag norms), replicated on
  every core.
* EA layers: bf16 matmuls, LayerNorm fused as mean/E[x2] reductions + one
  Relu activation with AP scale/bias; only {Exp, Ln, Square} activation
  functions are used so a single act table set covers the whole kernel.

KLEVEL env (debug bisection): pool|ea|mgrm|nocc|full
"""

import os
import sys

for _p in ("/opt/trn_rl_repo", "/root/.axon_site/_ro/trn_rl_repo"):
    if _p not in sys.path:
        sys.path.insert(0, _p)

import numpy as np
import ml_dtypes

import concourse.bacc as bacc
import concourse.mybir as mybir
import concourse.tile as tile
from concourse import bass_utils
from concourse.masks import make_identity, make_upper_triangular, make_lower_triangular

F32 = mybir.dt.float32
BF16 = mybir.dt.bfloat16
FP8 = mybir.dt.float8e4
I32 = mybir.dt.int32
AX = mybir.AxisListType
OP = mybir.AluOpType
ACT = mybir.ActivationFunctionType
BFnp = ml_dtypes.bfloat16
F8np = ml_dtypes.float8_e4m3

NCORES = 8
N, NS, C, D = 512, 256, 21, 2048
NL = N // NCORES          # 64 rows per core
H1 = 1024
DIN = D + C               # 2069
S = 32                    # per-class sequence table length (max count is 18)
THR0, MOM, EPS, LN_EPS, MGRM_W = 0.1, 0.5, 1e-8, 1e-5, 1.0

# feature permutation: new index f holds original d = (f % 128) * 16 + f // 128
PERM = (np.arange(D) % 128) * 16 + np.arange(D) // 128


def _build():
    lv = os.environ.get("KLEVEL", "full")
    do_mgrm = lv in ("mgrm", "nocc", "full")
    do_ea = lv in ("ea", "nocc", "full")
    do_final = lv in ("nocc", "full")

    nc = bacc.Bacc("TRN2", target_bir_lowering=False, debug=False,
                   num_devices=NCORES)

    # ---------------- DRAM I/O ----------------
    # pair-major feature stream [128pair x 25 groups, 2048f]: pooling is
    # done on the PE as selector matmuls, keeping the DVE free for the
    # latency-critical MGRM phase-1 chain
    featp_dr = nc.dram_tensor("featP", [128, 25 * D], FP8,
                              kind="ExternalInput")
    enw_dr = nc.dram_tensor("Enw", [128, 25 * NL], FP8, kind="ExternalInput")
    lg_dr = nc.dram_tensor("logits_full", [N, C], F32, kind="ExternalInput")
    lgT_dr = nc.dram_tensor("logitsT_loc", [C, NL], BF16, kind="ExternalInput")
    lab_dr = nc.dram_tensor("labels_in", [NS], I32, kind="ExternalInput")
    dom_dr = nc.dram_tensor("dom_shard", [NL, 1], I32, kind="ExternalInput")
    sels_dr = nc.dram_tensor("sel_src", [NS, NL], F32, kind="ExternalInput")
    selt_dr = nc.dram_tensor("sel_tgt", [NS, NL], F32, kind="ExternalInput")
    w1_dr = nc.dram_tensor("W1p", [DIN, H1], BF16, kind="ExternalInput")
    w2_dr = nc.dram_tensor("W2in", [H1, H1], BF16, kind="ExternalInput")
    w3_dr = nc.dram_tensor("W3in", [H1, H1], BF16, kind="ExternalInput")
    wd_dr = nc.dram_tensor("Wdin", [H1, 1], BF16, kind="ExternalInput")
    b1_dr = nc.dram_tensor("b1in", [1, H1], BF16, kind="ExternalInput")
    b2_dr = nc.dram_tensor("b2in", [1, H1], BF16, kind="ExternalInput")
    b3_dr = nc.dram_tensor("b3in", [1, H1], BF16, kind="ExternalInput")
    bd_dr = nc.dram_tensor("bdin", [1, 1], BF16, kind="ExternalInput")
    out_dr = nc.dram_tensor("out_loss", [1, 2], F32, kind="ExternalOutput")

    featp3 = featp_dr.ap().rearrange("p (j x) -> p j x", j=25)  # x = 2048

    with tile.TileContext(nc) as tc:
        with (
            tc.tile_pool(name="consts", bufs=1) as cst,
            tc.tile_pool(name="insb", bufs=1) as insb,
            tc.tile_pool(name="featp", bufs=4) as featp,
            tc.tile_pool(name="persist", bufs=1) as per,
            tc.tile_pool(name="wpool", bufs=18) as wp,
            tc.tile_pool(name="wpool2", bufs=16) as wp2,
            tc.tile_pool(name="work", bufs=1) as wk,
            tc.tile_pool(name="pps", bufs=3, space="PSUM") as pps,
            tc.tile_pool(name="pph", bufs=1, space="PSUM") as pph,
            tc.tile_pool(name="dram", bufs=1, space="DRAM") as drp,
        ):
            # ---------------- constants ----------------
            id128 = cst.tile([128, 128], F32, tag="id128")
            make_identity(nc, id128[:])
            ut128 = cst.tile([128, 128], F32, tag="ut128")
            make_upper_triangular(nc, ut128[:], val=1.0, diag=True)
            lt128 = cst.tile([128, 128], F32, tag="lt128")
            make_lower_triangular(nc, lt128[:], val=1.0, diag=True)
            ones = cst.tile([128, 128], F32, tag="ones")
            nc.gpsimd.memset(ones[:], 1.0)
            onesbf = cst.tile([1, NL], BF16, tag="onesbf")
            nc.gpsimd.memset(onesbf[:], 1.0)
            iotaS = cst.tile([128, S], F32, tag="iotaS")   # 1..S per partition
            nc.gpsimd.iota(iotaS[:], [[1, S]], base=1, channel_multiplier=0,
                           allow_small_or_imprecise_dtypes=True)
            iota21 = cst.tile([128, C], F32, tag="iota21")  # 0..20
            nc.gpsimd.iota(iota21[:], [[1, C]], base=0, channel_multiplier=0,
                           allow_small_or_imprecise_dtypes=True)
            iota63 = cst.tile([128, 63], F32, tag="iota63")  # col index 0..62
            nc.gpsimd.iota(iota63[:], [[1, 63]], base=0, channel_multiplier=0,
                           allow_small_or_imprecise_dtypes=True)
            pc21 = cst.tile([128, 1], F32, tag="pc21")       # 21 + partition
            nc.gpsimd.iota(pc21[:], [[1, 1]], base=21, channel_multiplier=1,
                           allow_small_or_imprecise_dtypes=True)
            pc42 = cst.tile([128, 1], F32, tag="pc42")       # 42 + partition
            nc.gpsimd.iota(pc42[:], [[1, 1]], base=42, channel_multiplier=1,
                           allow_small_or_imprecise_dtypes=True)


            # ---------------- small input DMAs ----------------
            # sync queue: lg + labels first (phase-1 needs them), then stream
            lg_sb = insb.tile([128, 4 * C], F32, tag="lg")       # [128, 84]
            nc.sync.dma_start(
                lg_sb[:].rearrange("p (c l) -> p c l", c=4),
                lg_dr.ap().rearrange("(c p) l -> p c l", p=128))
            lab_i = insb.tile([128, 2], I32, tag="labi")
            nc.sync.dma_start(lab_i[:], lab_dr.ap().rearrange(
                "(c p) -> p c", p=128))
            lab_f = insb.tile([128, 2], F32, tag="labf")
            nc.vector.tensor_copy(lab_f[:], lab_i[:])

            # ============ MGRM phase 1: logits-only math ============
            eall = None
            counts = None
            if do_mgrm:
                # per-chunk softmax stats (chunks of 128 samples; 0,1=src 2,3=tgt)
                E_ch, OHP_ch, mlOHP_ch = [], [], []
                for ch in range(4):
                    lg_c = lg_sb[:, C * ch:C * (ch + 1)]
                    mx = wk.tile([128, 1], F32, tag=f"mx{ch}")
                    nc.vector.tensor_reduce(mx[:], lg_c, axis=AX.X, op=OP.max)
                    E = wk.tile([128, C], F32, tag=f"E{ch}")
                    nc.vector.tensor_scalar(out=E[:], in0=lg_c, scalar1=mx[:],
                                            scalar2=None, op0=OP.is_equal)
                    negmx = wk.tile([128, 1], F32, tag=f"nmx{ch}")
                    nc.vector.tensor_scalar_mul(negmx[:], mx[:], -1.0)
                    scr = wk.tile([128, C], F32, tag=f"scr{ch}")
                    den = wk.tile([128, 1], F32, tag=f"den{ch}")
                    nc.scalar.activation(scr[:], lg_c, ACT.Exp, bias=negmx[:],
                                         scale=1.0, accum_out=den[:])
                    ml = wk.tile([128, 1], F32, tag=f"ml{ch}")
                    nc.vector.reciprocal(ml[:], den[:])
                    E_ch.append(E)

                    # position of each sample within its domain's class list
                    dom = ch // 2   # 0 = src, 1 = tgt
                    P_ps = pps.tile([128, C], F32, tag="pps")
                    if ch % 2 == 0:
                        nc.tensor.matmul(P_ps[:], ut128[:], E[:], start=True,
                                         stop=True)
                    else:
                        nc.tensor.matmul(P_ps[:], ones[:], E_ch[2 * dom][:],
                                         start=True, stop=False)
                        nc.tensor.matmul(P_ps[:], ut128[:], E[:], start=False,
                                         stop=True)
                    pos = wk.tile([128, 1], F32, tag=f"pos{ch}")
                    posscr = wk.tile([128, C], F32, tag=f"poss{ch}")
                    nc.vector.tensor_tensor(posscr[:], P_ps[:], E[:],
                                            op=OP.mult)
                    nc.vector.tensor_reduce(pos[:], posscr[:], axis=AX.X,
                                            op=OP.add)
                    OHP = wk.tile([128, S], F32, tag=f"OHP{ch}")
                    nc.vector.tensor_scalar(out=OHP[:], in0=iotaS[:],
                                            scalar1=pos[:], scalar2=None,
                                            op0=OP.is_equal)
                    mlOHP = wk.tile([128, S], F32, tag=f"mlO{ch}")
                    nc.vector.tensor_scalar(out=mlOHP[:], in0=iotaS[:],
                                            scalar1=pos[:], scalar2=ml[:],
                                            op0=OP.is_equal, op1=OP.mult)
                    OHP_ch.append(OHP)
                    mlOHP_ch.append(mlOHP)

                sels_sb = insb.tile([128, 2 * NL], F32, tag="sels")
                nc.scalar.dma_start(
                    sels_sb[:].rearrange("p (c n) -> p c n", c=2),
                    sels_dr.ap().rearrange("(c p) n -> p c n", p=128))
                selt_sb = insb.tile([128, 2 * NL], F32, tag="selt")
                nc.scalar.dma_start(
                    selt_sb[:].rearrange("p (c n) -> p c n", c=2),
                    selt_dr.ap().rearrange("(c p) n -> p c n", p=128))

                # tables T[21, 2S]: cols 0..S-1 src, S..2S-1 tgt (dom-major)
                T_all = wk.tile([C, 2 * S], F32, tag="Tall")
                for dom in range(2):
                    T_ps = pps.tile([C, S], F32, tag="pps")
                    nc.tensor.matmul(T_ps[:], E_ch[2 * dom][:],
                                     mlOHP_ch[2 * dom][:], start=True,
                                     stop=False)
                    nc.tensor.matmul(T_ps[:], E_ch[2 * dom + 1][:],
                                     mlOHP_ch[2 * dom + 1][:], start=False,
                                     stop=True)
                    nc.vector.tensor_copy(T_all[:, S * dom:S * (dom + 1)],
                                          T_ps[:])

                # ------------- sequential threshold chain -------------
                # th history keeps the serial loop at 2 DVE ops/step; all 32
                # accept masks come from ONE vectorized is_ge afterwards
                T3 = T_all[:].rearrange("p (d s) -> p d s", d=2)
                thh = wk.tile([C, 2 * (S + 1)], F32, tag="thh")
                th3 = thh[:].rearrange("p (s d) -> p s d", s=S + 1)
                nc.gpsimd.memset(th3[:, 0, :], THR0)
                tmp = wk.tile([C, 2], F32, tag="chtmp")
                for s in range(S):
                    nc.vector.tensor_tensor(tmp[:], T3[:, :, s], th3[:, s, :],
                                            op=OP.add)
                    nc.vector.scalar_tensor_tensor(
                        out=th3[:, s + 1, :], in0=tmp[:], scalar=0.5,
                        in1=th3[:, s, :], op0=OP.mult, op1=OP.max)
                A = wk.tile([C, 2 * S], F32, tag="A")
                A3 = A[:].rearrange("p (d s) -> p d s", d=2)
                nc.vector.tensor_tensor(
                    A3[:].rearrange("p d s -> p s d"), T3[:].rearrange(
                        "p d s -> p s d"), th3[:, 0:S, :], op=OP.is_ge)

                # per-domain accept-index math -> wtab [21, 2S]
                # w_j = prod_{i>=j, accepted} (1/J_i): gated reverse product
                # scan (no Ln/Exp -> no act-table switches)
                gt = wk.tile([C, 2 * S], F32, tag="gt")
                gt3 = gt[:].rearrange("p (d s) -> p d s", d=2)
                for dom in range(2):
                    A_dom = A[:, S * dom:S * (dom + 1)]
                    A_T = wk.tile([S, C], F32, tag=f"AT{dom}")
                    at_ps = pps.tile([S, C], F32, tag="pps")
                    nc.tensor.transpose(at_ps[:], A_dom, id128[0:C, 0:C])
                    nc.vector.tensor_copy(A_T[:], at_ps[:])
                    J_ps = pps.tile([C, S], F32, tag="pps")
                    nc.tensor.matmul(J_ps[:], A_T[:], ut128[0:S, 0:S],
                                     start=True, stop=True)
                    jc = wk.tile([C, S], F32, tag=f"jc{dom}")
                    nc.vector.tensor_scalar(out=jc[:], in0=J_ps[:], scalar1=1.0,
                                            scalar2=None, op0=OP.max)
                    rj = wk.tile([C, S], F32, tag=f"rj{dom}")
                    nc.vector.reciprocal(rj[:], jc[:])
                    # g = A ? 1/J : 1  =  (1/J)*A + (1 - A)
                    am = wk.tile([C, S], F32, tag=f"am{dom}")
                    nc.vector.tensor_scalar(out=am[:], in0=A_dom, scalar1=-1.0,
                                            scalar2=1.0, op0=OP.mult,
                                            op1=OP.add)
                    gd = gt3[:, dom, :]
                    nc.vector.tensor_tensor(gd, rj[:], A_dom, op=OP.mult)
                    nc.vector.tensor_tensor(gd, gd, am[:], op=OP.add)
                # suffix product scan (Pool engine), then wtab = pw * A
                pw = wk.tile([C, 2 * S], F32, tag="pw")
                pw3 = pw[:].rearrange("p (d s) -> p d s", d=2)
                nc.vector.tensor_copy(pw3[:, :, S - 1], gt3[:, :, S - 1])
                for s in range(S - 2, -1, -1):
                    nc.vector.tensor_tensor(pw3[:, :, s], gt3[:, :, s],
                                            pw3[:, :, s + 1], op=OP.mult)
                wtab = wk.tile([C, 2 * S], F32, tag="wtab")
                nc.vector.tensor_tensor(wtab[:], pw[:], A[:], op=OP.mult)

                # ---------- per-sample weights, local gathers ----------
                # eall [64, 63+1]: cols 0:21 src EW, 21:42 tgt EW, 42:63 label
                eall = wk.tile([NL, 64], BF16, tag="eall")
                for dom in range(2):
                    sel = sels_sb if dom == 0 else selt_sb
                    ewl_ps = pps.tile([NL, C], F32, tag="pps")
                    for cc in range(2):
                        ch = 2 * dom + cc
                        ET = wk.tile([C, 128], F32, tag=f"ET{ch}")
                        et_ps = pps.tile([C, 128], F32, tag="pps")
                        nc.tensor.transpose(et_ps[:], E_ch[ch][:], id128[:])
                        nc.vector.tensor_copy(ET[:], et_ps[:])
                        G_ps = pps.tile([128, S], F32, tag="pps")
                        nc.tensor.matmul(G_ps[:], ET[:],
                                         wtab[:, S * dom:S * (dom + 1)],
                                         start=True, stop=True)
                        ws = wk.tile([128, 1], F32, tag=f"ws{ch}")
                        wscr = wk.tile([128, S], F32, tag=f"wscr{ch}")
                        nc.vector.tensor_tensor(wscr[:], G_ps[:],
                                                OHP_ch[ch][:], op=OP.mult)
                        nc.vector.tensor_reduce(ws[:], wscr[:], axis=AX.X,
                                                op=OP.add)
                        EW = wk.tile([128, C], F32, tag=f"EW{ch}")
                        nc.vector.tensor_scalar(out=EW[:], in0=E_ch[ch][:],
                                                scalar1=ws[:], scalar2=None,
                                                op0=OP.mult)
                        nc.tensor.matmul(ewl_ps[:],
                                         sel[:, NL * cc:NL * (cc + 1)],
                                         EW[:], start=(cc == 0), stop=(cc == 1))
                    nc.vector.tensor_copy(eall[:, C * dom:C * (dom + 1)],
                                          ewl_ps[:])

                # labels onehot + counts + local label gather
                elab_l_ps = pps.tile([NL, C], F32, tag="pps")
                cnt_ps = pps.tile([C, 1], F32, tag="pps")
                for cc in range(2):
                    Elab = wk.tile([128, C], F32, tag=f"Elab{cc}")
                    nc.vector.tensor_scalar(out=Elab[:], in0=iota21[:],
                                            scalar1=lab_f[:, cc:cc + 1],
                                            scalar2=None, op0=OP.is_equal)
                    nc.tensor.matmul(cnt_ps[:], Elab[:], ones[:, 0:1],
                                     start=(cc == 0), stop=(cc == 1))
                    nc.tensor.matmul(elab_l_ps[:],
                                     sels_sb[:, NL * cc:NL * (cc + 1)],
                                     Elab[:], start=(cc == 0), stop=(cc == 1))
                nc.vector.tensor_copy(eall[:, 2 * C:3 * C], elab_l_ps[:])
                counts = wk.tile([C, 1], F32, tag="counts")
                nc.vector.tensor_copy(counts[:], cnt_ps[:])

            # ============ feature stream + pool + EA-L1 + feat64 ============
            opTb = per.tile([128, 16 * NL], BF16, tag="opTb")  # x(1/49), bf16
            opTb3 = opTb[:].rearrange("p (g n) -> p g n", g=16)
            feat64 = per.tile([NL, D], BF16, tag="feat64")     # sample-major

            bounce = drp.tile([128, 1008], FP8, tag="bounce")
            prT = per.tile([128, 16 * 63], FP8, tag="prT")
            id128b = cst.tile([128, 128], BF16, tag="id128b")
            nc.vector.tensor_copy(id128b[:], id128[:])

            def prt_chunks(gs):
                for g in gs:
                    pr_ps = pps.tile([128, 63], F32, tag="pps")
                    nc.tensor.matmul(pr_ps[:], feat64[:, 128 * g:128 * (g + 1)],
                                     eall[:, 0:63], start=True, stop=True)
                    nc.vector.tensor_copy(prT[:, 63 * g:63 * (g + 1)], pr_ps[:])

            enw_sb = insb.tile([128, 25 * NL], FP8, tag="enw")
            nc.scalar.dma_start(enw_sb[:], enw_dr[:, :])
            lgT_sb = insb.tile([C, NL], BF16, tag="lgT")
            nc.scalar.dma_start(lgT_sb[:], lgT_dr[:, :])
            b1_sb = insb.tile([1, H1], BF16, tag="b1")
            nc.scalar.dma_start(b1_sb[:], b1_dr[:, :])
            b2_sb = insb.tile([1, H1], BF16, tag="b2")
            nc.scalar.dma_start(b2_sb[:], b2_dr[:, :])
            b3_sb = insb.tile([1, H1], BF16, tag="b3")
            nc.scalar.dma_start(b3_sb[:], b3_dr[:, :])
            bd_sb = insb.tile([1, 1], BF16, tag="bd")
            nc.scalar.dma_start(bd_sb[:], bd_dr[:, :])
            wd_sb = insb.tile([128, 8], BF16, tag="wd")
            nc.scalar.dma_start(
                wd_sb[:].rearrange("p (c o) -> p c o", c=8),
                wd_dr.ap().rearrange("(c p) o -> p c o", p=128))
            dom_i = insb.tile([NL, 1], I32, tag="domi")
            nc.scalar.dma_start(dom_i[:], dom_dr[:, :])
            dom_f = insb.tile([NL, 1], F32, tag="domf")
            nc.vector.tensor_copy(dom_f[:], dom_i[:])

            # --- stream: 25 pair-major chunks, pooled on the PE ---
            # pool_ps accumulates the full [64n, 2048f] pooled sums (4 psum
            # banks); L1 runs post-stream so its 2 banks never coexist with
            # a partial pool.
            pool_ps = pph.tile([NL, D], F32, tag="hbig")
            # 12 double-group chunks via fp8 DoubleRow matmuls (0.5 cyc/row:
            # out += E_j^T @ X_j + E_j+1^T @ X_j+1 in one instruction),
            # plus a final single group
            MM2 = mybir.MatmulPerfMode.DoubleRow
            for jp in range(12):
                ftp = featp.tile([128, 2 * D], FP8, tag="ftp")
                nc.sync.dma_start(
                    ftp[:].rearrange("p (g x) -> p g x", g=2),
                    featp3[:, 2 * jp:2 * jp + 2, :])
                ftp3 = ftp[:].rearrange("p (g x) -> p g x", g=2)
                lhs3 = enw_sb[:, 2 * NL * jp:2 * NL * (jp + 1)].rearrange(
                    "p (g n) -> p g n", g=2)
                for q in range(4):
                    nc.tensor.matmul(pool_ps[:, 512 * q:512 * (q + 1)],
                                     lhs3,
                                     ftp3[:, :, 512 * q:512 * (q + 1)],
                                     start=(jp == 0), stop=False,
                                     perf_mode=MM2)
            ftp = featp.tile([128, 2 * D], FP8, tag="ftp")
            nc.sync.dma_start(ftp[0:128, 0:D], featp3[:, 24, :])
            for q in range(4):
                nsl = slice(512 * q, 512 * (q + 1))
                nc.tensor.matmul(pool_ps[:, nsl],
                                 enw_sb[:, NL * 24:NL * 25],
                                 ftp[0:128, nsl],
                                 start=False, stop=True)

            # W1 + W2/W3 prefetch (act queue drains these mid-stream)
            w1ts = {}
            if do_ea:
                for g in range(16):
                    w1t = wp.tile([128, H1], BF16, tag="w")
                    w1ts[g] = w1t
                    nc.scalar.dma_start(w1t[:],
                                        w1_dr[128 * g:128 * (g + 1), :])
                wt16 = wp.tile([128, H1], BF16, tag="w")
                nc.scalar.dma_start(wt16[0:C, :], w1_dr[D:D + C, :])
                wpre = {}
                for wi, w_dr in enumerate((w2_dr, w3_dr)):
                    for kc in range(8):
                        wpt = wp2.tile([128, H1], BF16, tag="w2")
                        wpre[(wi, kc)] = wpt
                        # the fp8 stream is short: both W2 and W3 ride the
                        # sync queue after it, keeping the act queue clear
                        # for the post-pool copies
                        nc.sync.dma_start(wpt[:],
                                          w_dr[128 * kc:128 * (kc + 1), :])

            # pooled sums -> feat64 (sample-major, bf16) per psum quadrant,
            # with the AR-gating prototype partials pipelined right behind
            for q in range(4):
                nsl = slice(512 * q, 512 * (q + 1))
                if q % 2 == 0:
                    nc.scalar.activation(feat64[:, nsl], pool_ps[:, nsl],
                                         ACT.Copy)
                else:
                    nc.vector.tensor_copy(feat64[:, nsl], pool_ps[:, nsl])
                if do_mgrm:
                    for g2 in range(4 * q, 4 * q + 4, 2):
                        pr2_ps = pps.tile([128, 126], F32, tag="pps")
                        for k in range(2):
                            g = g2 + k
                            nc.tensor.matmul(pr2_ps[:, 63 * k:63 * (k + 1)],
                                             feat64[:, 128 * g:128 * (g + 1)],
                                             eall[:, 0:63], start=True,
                                             stop=True)
                        if (g2 // 2) % 2 == 0:
                            nc.vector.tensor_copy(
                                prT[:, 63 * g2:63 * (g2 + 2)], pr2_ps[:])
                        else:
                            nc.scalar.activation(
                                prT[:, 63 * g2:63 * (g2 + 2)], pr2_ps[:],
                                ACT.Copy)
            if do_mgrm:
                # SWDGE on the Pool queue: the completion sem is local to the
                # engine that fires the ReduceScatter right after
                nc.gpsimd.dma_start(bounce[:], prT[:])
            # opT-orientation bf16 copies for the EA layer-1 lhsT
            for g in range(16):
                tp2_ps = pps.tile([128, NL], BF16, tag="pps")
                nc.tensor.transpose(tp2_ps[:],
                                    feat64[:, 128 * g:128 * (g + 1)],
                                    id128b[0:NL, 0:NL])
                nc.scalar.activation(opTb3[:, g, :], tp2_ps[:], ACT.Copy,
                                     scale=1.0 / 49.0)
            if do_ea:
                h_big = pph.tile([NL, D], F32, tag="hbig")
                h_ps = h_big[:, 0:H1]
                for half in range(2):
                    nsl = slice(512 * half, 512 * (half + 1))
                    nc.tensor.matmul(h_ps[:, nsl], onesbf[:], b1_sb[:, nsl],
                                     start=True, stop=False)
                for half in range(2):
                    nsl = slice(512 * half, 512 * (half + 1))
                    nc.tensor.matmul(h_ps[:, nsl], lgT_sb[:], wt16[0:C, nsl],
                                     start=False, stop=False)
                for g in range(16):
                    for half in range(2):
                        nsl = slice(512 * half, 512 * (half + 1))
                        nc.tensor.matmul(h_ps[:, nsl], opTb3[:, g, :],
                                         w1ts[g][:, nsl], start=False,
                                         stop=(g == 15))

            # ============ EA layers ============
            U16 = mybir.dt.uint16

            def rsqrt16(y, x, shape, tg):
                """y = x^-0.5 on DVE: bf16 bit-trick seed + 2 Newton steps.
                (no rsqrt opcode on DVE; act-table Sqrt would thrash the
                single {exp,ln,square} table set.  The DVE ALU datapath is
                fp32 internally so only 16-bit ints survive bit tricks.)"""
                xb = wk.tile(shape, BF16, tag=f"rsqb{tg}")
                nc.vector.tensor_copy(xb[:], x[:])
                t16 = wk.tile(shape, U16, tag=f"rsqs{tg}")
                nc.vector.tensor_scalar(out=t16[:], in0=xb[:].bitcast(U16),
                                        scalar1=1, scalar2=None,
                                        op0=OP.logical_shift_right)
                y16 = wk.tile(shape, U16, tag=f"rsqy{tg}")
                nc.vector.tensor_scalar(out=y16[:], in0=t16[:], scalar1=-1.0,
                                        scalar2=float(0x5F37), op0=OP.mult,
                                        op1=OP.add)
                t1 = wk.tile(shape, F32, tag=f"rsqt{tg}")
                nc.vector.tensor_copy(y[:], y16[:].bitcast(BF16))
                for _ in range(2):
                    nc.vector.tensor_tensor(t1[:], y[:], y[:], op=OP.mult)
                    nc.vector.tensor_tensor(t1[:], t1[:], x[:], op=OP.mult)
                    nc.vector.tensor_scalar(out=t1[:], in0=t1[:], scalar1=-0.5,
                                            scalar2=1.5, op0=OP.mult,
                                            op1=OP.add)
                    nc.vector.tensor_tensor(y[:], y[:], t1[:], op=OP.mult)

            def ln_relu(h_in, scaled):
                """relu(layernorm(h_in)) -> f32 [NL, H1].

                scaled=False drops the 1/std factor: relu(c*x) = c*relu(x)
                and the NEXT LayerNorm removes any per-row scale exactly
                (biases are zero in this problem), so only the last LN
                before the sigmoid needs the true rstd."""
                musum = wk.tile([NL, 1], F32, tag="mu")
                nc.vector.tensor_reduce(musum[:], h_in[:], axis=AX.X,
                                        op=OP.add)
                mu = wk.tile([NL, 1], F32, tag="mus")
                nc.vector.tensor_scalar_mul(mu[:], musum[:], 1.0 / H1)
                h = wk.tile([NL, H1], F32, tag="h")
                if not scaled:
                    nmu = wk.tile([NL, 1], F32, tag="nmu")
                    nc.vector.tensor_scalar_mul(nmu[:], musum[:], -1.0 / H1)
                    nc.scalar.activation(h[:], h_in[:], ACT.Relu, bias=nmu[:])
                    return h
                sqs = wk.tile([NL, H1], F32, tag="sqs")
                q = wk.tile([NL, 1], F32, tag="q")
                nc.scalar.activation(sqs[:], h_in[:], ACT.Square,
                                     accum_out=q[:])
                nmu2e = wk.tile([NL, 1], F32, tag="nmu2e")
                nc.vector.tensor_scalar(out=nmu2e[:], in0=mu[:], scalar1=mu[:],
                                        scalar2=-1.0, op0=OP.mult, op1=OP.mult)
                nc.vector.tensor_scalar(out=nmu2e[:], in0=nmu2e[:],
                                        scalar1=float(LN_EPS), scalar2=None,
                                        op0=OP.add)
                ve = wk.tile([NL, 1], F32, tag="ve")
                nc.vector.tensor_scalar(out=ve[:], in0=q[:], scalar1=1.0 / H1,
                                        scalar2=nmu2e[:], op0=OP.mult,
                                        op1=OP.add)
                rstd = wk.tile([NL, 1], F32, tag="rstd")
                rsqrt16(rstd, ve, [NL, 1], "ln")
                nb = wk.tile([NL, 1], F32, tag="nb")
                nc.vector.tensor_scalar(out=nb[:], in0=mu[:], scalar1=rstd[:],
                                        scalar2=-1.0, op0=OP.mult, op1=OP.mult)
                nc.scalar.activation(h[:], h_in[:], ACT.Relu, scale=rstd[:],
                                     bias=nb[:])
                return h

            def transpose_h(h):
                """[NL, H1] f32 -> [128, 8*NL] bf16 (chunk j = features 128j..)"""
                hT = wk.tile([128, 8 * NL], BF16, tag="hT")
                for half in range(2):
                    ht_ps = pps.tile([128, 4 * NL], F32, tag="pps")
                    for j in range(4):
                        jj = 4 * half + j
                        nc.tensor.transpose(ht_ps[:, NL * j:NL * (j + 1)],
                                            h[:, 128 * jj:128 * (jj + 1)],
                                            id128[0:NL, 0:NL])
                    nc.vector.tensor_copy(
                        hT[:, 4 * NL * half:4 * NL * (half + 1)], ht_ps[:])
                return hT

            lea_ps = None
            if do_ea:
                h = ln_relu(h_ps[:], scaled=True)
                for li_w, (w_dr, b_sb) in enumerate(((w2_dr, b2_sb),
                                                     (w3_dr, b3_sb))):
                    hT = transpose_h(h)
                    h_big = pph.tile([NL, D], F32, tag="hbig")
                    h_ps = h_big[:, 0:H1]
                    for half in range(2):
                        nsl = slice(512 * half, 512 * (half + 1))
                        nc.tensor.matmul(h_ps[:, nsl], onesbf[:], b_sb[:, nsl],
                                         start=True, stop=False)
                    for kc in range(8):
                        wt = wpre[(li_w, kc)]
                        for half in range(2):
                            nsl = slice(512 * half, 512 * (half + 1))
                            nc.tensor.matmul(h_ps[:, nsl],
                                             hT[:, NL * kc:NL * (kc + 1)],
                                             wt[:, nsl],
                                             start=False,
                                             stop=(kc == 7))
                    h = ln_relu(h_ps[:], scaled=True)

                h3T = transpose_h(h)
                zd_ps = pps.tile([NL, 1], F32, tag="pps")
                nc.tensor.matmul(zd_ps[:], onesbf[:], bd_sb[:],
                                 start=True, stop=False)
                for kc in range(8):
                    nc.tensor.matmul(zd_ps[:], h3T[:, NL * kc:NL * (kc + 1)],
                                     wd_sb[:, kc:kc + 1], start=False,
                                     stop=(kc == 7))
                # z = sigmoid(zd) = 1/(1+exp(-zd))
                enz = wk.tile([NL, 1], F32, tag="enz")
                nc.scalar.activation(enz[:], zd_ps[:], ACT.Exp, scale=-1.0)
                zden = wk.tile([NL, 1], F32, tag="zden")
                nc.vector.tensor_scalar(out=zden[:], in0=enz[:], scalar1=1.0,
                                        scalar2=None, op0=OP.add)
                z = wk.tile([NL, 1], F32, tag="z")
                nc.vector.reciprocal(z[:], zden[:])
                # softplus(-z) = ln(1 + exp(-z))
                emz = wk.tile([NL, 1], F32, tag="emz")
                nc.scalar.activation(emz[:], z[:], ACT.Exp, scale=-1.0)
                sp = wk.tile([NL, 1], F32, tag="sp")
                nc.scalar.activation(sp[:], emz[:], ACT.Ln,
                                     bias=ones[0:NL, 0:1])
                omy = wk.tile([NL, 1], F32, tag="omy")
                nc.vector.tensor_scalar(out=omy[:], in0=dom_f[:], scalar1=-1.0,
                                        scalar2=1.0, op0=OP.mult, op1=OP.add)
                li_t = wk.tile([NL, 1], F32, tag="li")
                nc.vector.scalar_tensor_tensor(out=li_t[:], in0=z[:],
                                               scalar=omy[:], in1=sp[:],
                                               op0=OP.mult, op1=OP.add)
                # per-core loss_ea partial: reduced on the HOST (data-parallel
                # gather), so the AllReduce is not gated on the EA branch
                lea_ps = pps.tile([1, 1], F32, tag="pps")
                nc.tensor.matmul(lea_ps[:], li_t[:], ones[0:NL, 0:1],
                                 start=True, stop=True)


            if do_final:
                # ---- cross-core reduce: ReduceScatter + AllGather ----
                # (cheaper than AllReduce: no 1.875x single-instr penalty)
                bounce_out = drp.tile([128, 1008], FP8, tag="bounce_out",
                                      addr_space="Shared")
                if lv == "nocc":
                    nc.sync.dma_start(bounce_out[:], bounce[:])
                else:
                    bmid = drp.tile([16, 1008], FP8, tag="bmid")
                    nc.gpsimd.collective_compute(
                        "ReduceScatter", OP.add,
                        replica_groups=[list(range(NCORES))],
                        ins=[bounce[:].opt()], outs=[bmid[:].opt()])
                    nc.gpsimd.collective_compute(
                        "AllGather", OP.bypass,
                        replica_groups=[list(range(NCORES))],
                        ins=[bmid[:].opt()], outs=[bounce_out[:].opt()])
                XT = per.tile([128, 16 * 63], FP8, tag="XT")
                nc.gpsimd.dma_start(XT[:], bounce_out[:])

                # ---------------- final (replicated) ----------------
                # full Gram G[63,63] of the stacked [ps|pt|bw] prototypes;
                # diag = squared norms, blocks = cross inner products
                G_ps = pps.tile([63, 63], F32, tag="pps")
                for g in range(16):
                    base = 63 * g
                    nc.tensor.matmul(G_ps[:], XT[:, base:base + 63],
                                     XT[:, base:base + 63],
                                     start=(g == 0), stop=(g == 15))
                G_sb = wk.tile([63, 63], F32, tag="Gsb")
                nc.vector.tensor_copy(G_sb[:], G_ps[:])
                # realign pt rows (21..41) and bw rows (42..62) down to
                # partitions 0..20 with shifted-identity matmuls
                Gpt_ps = pps.tile([C, 63], F32, tag="pps")
                nc.tensor.matmul(Gpt_ps[:], id128[0:63, C:2 * C], G_sb[:],
                                 start=True, stop=True)
                Gbw_ps = pps.tile([C, 63], F32, tag="pps")
                nc.tensor.matmul(Gbw_ps[:], id128[0:63, 2 * C:3 * C], G_sb[:],
                                 start=True, stop=True)
                # diag extraction masks on partitions 0..20
                mpt = wk.tile([C, 63], F32, tag="mpt")
                nc.vector.tensor_scalar(out=mpt[:], in0=iota63[0:C, :],
                                        scalar1=pc21[0:C, :], scalar2=None,
                                        op0=OP.is_equal)
                mbw = wk.tile([C, 63], F32, tag="mbw")
                nc.vector.tensor_scalar(out=mbw[:], in0=iota63[0:C, :],
                                        scalar1=pc42[0:C, :], scalar2=None,
                                        op0=OP.is_equal)
                # d3 cols: 0 = ps, 1 = pt, 2 = bw squared norms
                d3 = wk.tile([C, 3], F32, tag="d3")
                dsc = wk.tile([C, 63], F32, tag="dsc")
                nc.vector.tensor_tensor(dsc[:], G_sb[0:C, :], id128[0:C, 0:63],
                                        op=OP.mult)
                nc.vector.tensor_reduce(d3[:, 0:1], dsc[:], axis=AX.X,
                                        op=OP.add)
                nc.vector.tensor_tensor(dsc[:], Gpt_ps[:], mpt[:], op=OP.mult)
                nc.vector.tensor_reduce(d3[:, 1:2], dsc[:], axis=AX.X,
                                        op=OP.add)
                nc.vector.tensor_tensor(dsc[:], Gbw_ps[:], mbw[:], op=OP.mult)
                nc.vector.tensor_reduce(d3[:, 2:3], dsc[:], axis=AX.X,
                                        op=OP.add)
                # rn3 = 1/max(sqrt(d), eps) = max(d, eps^2)^-0.5
                dmx3 = wk.tile([C, 3], F32, tag="dmx3")
                nc.vector.tensor_scalar(out=dmx3[:], in0=d3[:],
                                        scalar1=float(EPS) ** 2, scalar2=None,
                                        op0=OP.max)
                rn3 = wk.tile([C, 3], F32, tag="rn3")
                rsqrt16(rn3, dmx3, [C, 3], "rn")

                cosP = wk.tile([C, C], F32, tag="cosP")
                nc.vector.tensor_scalar(out=cosP[:],
                                        in0=G_sb[0:C, C:2 * C],
                                        scalar1=rn3[:, 0:1], scalar2=None,
                                        op0=OP.mult)
                cosB = wk.tile([C, C], F32, tag="cosB")
                nc.vector.tensor_scalar(out=cosB[:], in0=Gbw_ps[:, C:2 * C],
                                        scalar1=rn3[:, 2:3], scalar2=None,
                                        op0=OP.mult)
                absd = wk.tile([C, C], F32, tag="absd")
                nc.vector.tensor_tensor(absd[:], cosB[:], cosP[:],
                                        op=OP.subtract)
                nc.scalar.activation(absd[:], absd[:], ACT.Abs)
                # cem = (counts > 0) with class 0 zeroed; wv = rn_t * cem
                cem = wk.tile([C, 1], F32, tag="cem")
                nc.vector.tensor_scalar(out=cem[:], in0=counts[:], scalar1=0.0,
                                        scalar2=None, op0=OP.is_gt)
                nc.gpsimd.memset(cem[0:1, :], 0.0)
                wv = wk.tile([C, 1], F32, tag="wv")
                nc.vector.tensor_tensor(wv[:], rn3[:, 1:2], cem[:],
                                        op=OP.mult)
                # sum over j = 1..20 only (reference drops row/col 0)
                nc.gpsimd.memset(absd[0:1, :], 0.0)
                s1_ps = pps.tile([C, 1], F32, tag="pps")
                nc.tensor.matmul(s1_ps[:], absd[:], ones[0:C, 0:1],
                                 start=True, stop=True)
                s1 = wk.tile([C, 1], F32, tag="s1")
                nc.vector.tensor_copy(s1[:], s1_ps[:])
                tot_ps = pps.tile([1, 1], F32, tag="pps")
                nc.tensor.matmul(tot_ps[:], s1[:], wv[:], start=True, stop=True)
                nm_ps = pps.tile([1, 1], F32, tag="pps")
                nc.tensor.matmul(nm_ps[:], cem[:], ones[0:C, 0:1],
                                 start=True, stop=True)
                nm_sb = wk.tile([1, 1], F32, tag="nmsb")
                nc.vector.tensor_copy(nm_sb[:], nm_ps[:])
                rnm = wk.tile([1, 1], F32, tag="rnm")
                nc.vector.reciprocal(rnm[:], nm_sb[:])

                res = wk.tile([1, 2], F32, tag="res")
                nc.vector.tensor_scalar(out=res[:, 0:1], in0=tot_ps[:],
                                        scalar1=rnm[:],
                                        scalar2=MGRM_W / (C - 1.0),
                                        op0=OP.mult, op1=OP.mult)
                if do_ea:
                    nc.vector.tensor_copy(res[:, 1:2], lea_ps[:])
                else:
                    nc.gpsimd.memset(res[:, 1:2], 0.0)
                nc.sync.dma_start(out_dr[:, :], res[:])
            else:
                res = wk.tile([1, 2], F32, tag="res")
                if lv == "pool":
                    nc.vector.tensor_copy(res[:], feat64[0:1, 0:2])
                elif lv == "ea":
                    le = wk.tile([1, 1], F32, tag="leadbg")
                    nc.vector.tensor_copy(le[:], lea_ps[:])
                    nc.vector.tensor_scalar(out=res[:, 0:1], in0=le[:],
                                            scalar1=1.0, scalar2=None,
                                            op0=OP.mult)
                    nc.vector.tensor_copy(res[:, 1:2], feat64[0:1, 0:1])
                else:
                    nc.vector.tensor_copy(res[:, 0:1], prT[0:1, 0:1])
                    nc.vector.tensor_copy(res[:, 1:2], feat64[0:1, 0:1])
                nc.sync.dma_start(out_dr[:, :], res[:])

    nc.compile()
    return nc


_NC_CACHE = {}
_last_in_maps = None


def _prep_in_maps(inputs):
    feats = np.asarray(inputs["ins_features"], np.float32)
    logits = np.ascontiguousarray(inputs["class_logits"], dtype=np.float32)
    labels = np.ascontiguousarray(inputs["labels"], dtype=np.int32)
    dom = np.ascontiguousarray(inputs["domain_labels"], dtype=np.int32)
    W1 = np.asarray(inputs["W1"], np.float32)
    W1p = np.ascontiguousarray(
        np.concatenate([W1[:D][PERM], W1[D:]], axis=0).astype(BFnp))
    W2b = np.ascontiguousarray(np.asarray(inputs["W2"], np.float32).astype(BFnp))
    W3b = np.ascontiguousarray(np.asarray(inputs["W3"], np.float32).astype(BFnp))
    Wdb = np.ascontiguousarray(np.asarray(inputs["Wd"], np.float32).astype(BFnp))
    b1 = np.asarray(inputs["b1"], np.float32).astype(BFnp).reshape(1, H1)
    b2 = np.asarray(inputs["b2"], np.float32).astype(BFnp).reshape(1, H1)
    b3 = np.asarray(inputs["b3"], np.float32).astype(BFnp).reshape(1, H1)
    bd = np.asarray(inputs["bd"], np.float32).astype(BFnp).reshape(1, 1)

    # fp8-e4m3 stream; d = 16p+g so feats_8 is [n, 128p, 16g, 49w]
    feats_8 = feats.reshape(N, 128, 16, 49).astype(F8np)
    logits_bf = logits.astype(BFnp)

    # selector for the PE-pooled half: pair q = 50n + w -> sample n
    # (w = 49 is padding; its data rows are zero so E there is harmless)
    qn = np.arange(128)
    Enw = np.zeros((25, 128, NL), F8np)
    for j in range(25):
        Enw[j, qn, (128 * j + qn) // 50] = 1.0
    Enw_flat = np.ascontiguousarray(Enw.transpose(1, 0, 2)).reshape(128, -1)

    in_maps = []
    for k in range(NCORES):
        r0 = NL * k
        sel_s = np.zeros((NS, NL), np.float32)
        sel_t = np.zeros((NS, NL), np.float32)
        if r0 + NL <= NS:
            sel_s[np.arange(r0, r0 + NL), np.arange(NL)] = 1.0
        else:
            sel_t[np.arange(r0 - NS, r0 - NS + NL), np.arange(NL)] = 1.0
        # pair-major [(n, w) padded to 50, f = 128g+p]
        pe = np.zeros((NL, 50, D), F8np)
        pe[:, 0:49, :] = feats_8[r0:r0 + NL].transpose(0, 3, 2, 1) \
            .reshape(NL, 49, D)
        featP = np.ascontiguousarray(
            pe.reshape(25, 128, D).transpose(1, 0, 2)).reshape(128, -1)
        in_maps.append({
            "featP": featP,
            "Enw": Enw_flat,
            "logits_full": logits,
            "logitsT_loc": np.ascontiguousarray(logits_bf[r0:r0 + NL].T),
            "labels_in": labels,
            "dom_shard": np.ascontiguousarray(dom[r0:r0 + NL].reshape(NL, 1)),
            "sel_src": sel_s,
            "sel_tgt": sel_t,
            "W1p": W1p, "W2in": W2b, "W3in": W3b, "Wdin": Wdb,
            "b1in": b1, "b2in": b2, "b3in": b3, "bdin": bd,
        })
    return in_maps


def kernel(**inputs) -> np.ndarray:
    lv = os.environ.get("KLEVEL", "full")
    if _NC_CACHE.get("lv") != lv:
        _NC_CACHE.clear()
        _NC_CACHE["nc"] = _build()
        _NC_CACHE["lv"] = lv
    nc = _NC_CACHE["nc"]
    in_maps = _prep_in_maps(inputs)
    global _last_in_maps
    _last_in_maps = in_maps
    res = bass_utils.run_bass_kernel_spmd(nc, in_maps,
                                          core_ids=list(range(NCORES)))
    out0 = res.results[0]["out_loss"].reshape(2).astype(np.float32)
    if lv in ("full", "nocc"):
        # loss_ea is returned as per-core partial sums (data-parallel
        # gather): combine on the host
        lea = sum(float(r["out_loss"].reshape(2)[1]) for r in res.results)
        return np.array([out0[0], lea / N], np.float32)
    return out0
